# revision 17
# baseline (speedup 1.0000x reference)
"""EnhancedMambaFormerBlock Trainium2 kernel.

Data-parallel over batch: 8 batch elements -> 8 NeuronCores, no collectives.
Per core the full block (LN -> Mamba(conv + selective scan) -> LN -> attention
-> gated fusion -> LN -> FFN) runs with a transposed-layout compute chain:

 - Matmuls run in fp32r (full PE speed at N>=256, ~1e-4 rounding) or bf16
   where precision allows (dt_proj / x_proj / ffn).
 - The selective scan uses the DVE `tensor_tensor_scan` instruction
   (state = dA * state + dBx along time), one scan per
   (state n, 128-channel block, 512-time segment); decay factors come from
   ScalarE exp(A_n * delta); the C-contraction over n uses GpSimd multiplies
   + PE identity-matmul accumulation in PSUM.
 - Attention scores are computed in BOTH layouts ([tq,tk] for softmax /
   attn_w output, [tk,tq] for attn@V); exp + row sums via ScalarE accum_out.

Self-contained: hardcodes all shapes from the problem spec.
"""
import sys
from contextlib import ExitStack

import numpy as np

sys.path.insert(0, "/opt/trn_rl_repo")

import concourse.tile as tile  # noqa: E402
from concourse import bacc, mybir  # noqa: E402

F32 = mybir.dt.float32
F32R = mybir.dt.float32r
BF16 = mybir.dt.bfloat16
AF = mybir.ActivationFunctionType
OP = mybir.AluOpType

DM = 512
DS = 16
DCONV = 4
DI = 1024
H = 8
DK = 64
DFF = 2048
EPS = 1e-5
P = 128

KC = DM // P   # 4  dmodel chunks
KI = DI // P   # 8  d_inner chunks
KF = DFF // P  # 16 dff chunks


def build_program(L=1024, debug=False):
    NT = L // P
    SEG = min(512, L)
    TH = L // SEG

    nc = bacc.Bacc("TRN2", target_bir_lowering=False, debug=False)

    x_in = nc.dram_tensor("x_in", [L, DM], F32, kind="ExternalInput")
    out_x = nc.dram_tensor("out_x", [L, DM], F32, kind="ExternalOutput")
    attn_w_out = nc.dram_tensor("attn_w_out", [H, L, L], F32, kind="ExternalOutput")
    if debug:
        dbg_m = nc.dram_tensor("dbg_m", [L, DM], F32, kind="ExternalOutput")
        dbg_a = nc.dram_tensor("dbg_a", [L, DM], F32, kind="ExternalOutput")
        dbg_g = nc.dram_tensor("dbg_g", [L, DM], F32, kind="ExternalOutput")
        dbg_x3 = nc.dram_tensor("dbg_x3", [L, DM], F32, kind="ExternalOutput")
        dbg_h1 = nc.dram_tensor("dbg_h1", [DFF, L], F32, kind="ExternalOutput")

    wd = {}
    for name, shape in [
        ("in_proj_w", [DM, 2 * DI]), ("conv_w", [DI, DCONV]), ("conv_b", [DI]),
        ("x_proj_w", [DI, 2 * DS]), ("dt_proj_w", [DI, DI]), ("dt_proj_b", [DI]),
        ("A_log", [DS]), ("D", [DI]), ("out_proj_w", [DI, DM]),
        ("wq_w", [DM, DM]), ("wq_b", [DM]), ("wk_w", [DM, DM]), ("wk_b", [DM]),
        ("wv_w", [DM, DM]), ("wv_b", [DM]), ("wo_w", [DM, DM]), ("wo_b", [DM]),
        ("attn_ln_g", [DM]), ("attn_ln_b", [DM]),
        ("ffn_w1", [DM, DFF]), ("ffn_b1", [DFF]), ("ffn_w2", [DFF, DM]), ("ffn_b2", [DM]),
        ("norm1_g", [DM]), ("norm1_b", [DM]), ("norm2_g", [DM]), ("norm2_b", [DM]),
        ("norm3_g", [DM]), ("norm3_b", [DM]),
        ("gate_w", [2 * DM, DM]), ("gate_b", [DM]),
        ("ident", [P, P]), ("ones_row", [1, P]),
    ]:
        wd[name] = nc.dram_tensor(name, shape, F32, kind="ExternalInput")

    with tile.TileContext(nc) as tc, ExitStack() as top:
        persist = top.enter_context(tc.tile_pool(name="persist", bufs=1))
        dscratch = top.enter_context(tc.tile_pool(name="dscratch", bufs=1, space="DRAM"))
        pwork = top.enter_context(tc.tile_pool(name="pwork", bufs=2))
        pstat = top.enter_context(tc.tile_pool(name="pstat", bufs=4))
        ptr = top.enter_context(tc.tile_pool(name="ptr", bufs=2, space="PSUM"))

        # ---------------- consts ----------------
        ident_f = persist.tile([P, P], F32, tag="ident_f")
        nc.sync.dma_start(ident_f[:], wd["ident"][:])
        ident_r = persist.tile([P, P], F32R, tag="ident_r")
        nc.sync.dma_start(ident_r[:], wd["ident"][:].bitcast(F32R))
        ones_r = persist.tile([1, P], F32R, tag="ones_r")
        nc.sync.dma_start(ones_r[:], wd["ones_row"][:].bitcast(F32R))
        eps_t = persist.tile([P, 1], F32, tag="eps")
        nc.vector.memset(eps_t[:], EPS)

        alog_rep = persist.tile([P, DS], F32, tag="alog_rep")
        nc.sync.dma_start(alog_rep[:], wd["A_log"][None, :].partition_broadcast(P))
        A_rep = persist.tile([P, DS], F32, tag="A_rep")
        nc.scalar.activation(A_rep[:], alog_rep[:], AF.Exp)
        nc.vector.tensor_scalar_mul(A_rep[:], A_rep[:], -1.0)

        def ppart(name, cols, tag):
            t = persist.tile([P, cols], F32, tag=tag)
            nc.sync.dma_start(t[:], wd[name][:].rearrange("(o p) -> p o", p=P))
            return t

        conv_b_sb = ppart("conv_b", KI, "conv_b")
        dt_b_sb = ppart("dt_proj_b", KI, "dt_b")
        D_sb = ppart("D", KI, "D_sb")
        g1_sb = ppart("norm1_g", KC, "g1")
        b1n_sb = ppart("norm1_b", KC, "b1n")
        g2_sb = ppart("norm2_g", KC, "g2")
        b2n_sb = ppart("norm2_b", KC, "b2n")
        g3_sb = ppart("norm3_g", KC, "g3")
        b3n_sb = ppart("norm3_b", KC, "b3n")
        ga_sb = ppart("attn_ln_g", KC, "ga")
        ba_sb = ppart("attn_ln_b", KC, "ba")
        wqb_sb = ppart("wq_b", KC, "wqb")
        wkb_sb = ppart("wk_b", KC, "wkb")
        fb1_sb = ppart("ffn_b1", KF, "fb1")
        conv_w_sb = persist.tile([P, KI, DCONV], F32, tag="conv_w")
        nc.sync.dma_start(conv_w_sb[:], wd["conv_w"][:].rearrange("(o p) k -> p o k", p=P))

        def vrep(name, tag):
            t = persist.tile([P, DM], F32, tag=tag)
            nc.sync.dma_start(t[:], wd[name][None, :].partition_broadcast(P))
            return t

        g2_rep = vrep("norm2_g", "g2rep")
        b2_rep = vrep("norm2_b", "b2rep")
        ga_rep = vrep("attn_ln_g", "garep")
        ba_rep = vrep("attn_ln_b", "barep")

        def brow(name, n, tag):
            t = persist.tile([1, n], F32R, tag=tag)
            nc.sync.dma_start(t[:], wd[name][None, :].bitcast(F32R))
            return t

        wvb_row = brow("wv_b", DM, "wvb_row")
        wob_row = brow("wo_b", DM, "wob_row")
        gateb_row = brow("gate_b", DM, "gateb_row")
        fb2_row = brow("ffn_b2", DM, "fb2_row")

        recip_dram = dscratch.tile([H, L, 1], F32, tag="recip_dram")
        x_acc = persist.tile([P, NT, DM], F32, tag="x_acc")
        nc.sync.dma_start(x_acc[:], x_in[:].rearrange("(n p) d -> p n d", p=P))



        # ---------------- helpers ----------------
        def ln_stats(xt):
            st = pstat.tile([P, 6], F32, tag="st6")
            nc.vector.bn_stats(st[:], xt)
            mv = pstat.tile([P, 2], F32, tag="mv2")
            nc.vector.bn_aggr(mv[:], st[:])
            rstd = pstat.tile([P, 1], F32, tag="rstd")
            nc.scalar.activation(rstd[:], mv[:, 1:2], AF.Sqrt, bias=eps_t[:])
            nc.vector.reciprocal(rstd[:], rstd[:])
            return mv[:, 0:1], rstd

        def center_scale(src):
            mu, rstd = ln_stats(src)
            cs = pwork.tile([P, DM], F32, tag="cs_t")
            nc.vector.tensor_scalar(
                out=cs[:], in0=src, scalar1=mu, scalar2=rstd,
                op0=OP.subtract, op1=OP.mult)
            return cs

        def transpose_apply(cs_t, it, g_col, b_col, dstT):
            for jc in range(KC):
                ps = ptr.tile([P, P], F32, tag="tr_ps")
                nc.tensor.transpose(ps[:], cs_t[:, jc * P:(jc + 1) * P], ident_f[:])
                nc.scalar.activation(
                    dstT[:, jc, it * P:(it + 1) * P], ps[:], AF.Identity,
                    scale=g_col[:, jc:jc + 1], bias=b_col[:, jc:jc + 1])

        # ================= mamba =================
        mid_cm = tc.tile_pool(name="mid", bufs=1)
        mid = mid_cm.__enter__()
        m_outT = mid.tile([P, KC, L], F32R, tag="m_outT")
        m_out = mid.tile([P, NT, DM], F32, tag="m_out")
        with tc.tile_pool(name="mamba", bufs=1) as mpool:
            xcT = mpool.tile([P, KI, L], BF16, tag="xcT")
            BCT_dram = dscratch.tile([2 * DS, L], F32, tag="BCT_dram")
            sresT = mpool.tile([P, KI, L], BF16, tag="sresT")
            BCT = mpool.tile([2 * DS, L], F32, tag="BCT")

            with tc.tile_pool(name="inproj", bufs=1) as ipool, \
                    tc.tile_pool(name="iwork", bufs=2) as iwork, \
                    tc.tile_pool(name="ps_ip", bufs=2, space="PSUM") as ps_ip:
                xn1T = ipool.tile([P, KC, L], F32R, tag="xn1T")
                for it in range(NT):
                    cs = center_scale(x_acc[:, it, :])
                    transpose_apply(cs[:], it, g1_sb, b1n_sb, xn1T)

                xmT_pad = ipool.tile([P, KI, L + DCONV - 1], F32, tag="xmT_pad")
                for oi in range(KI):
                    nc.vector.memset(xmT_pad[:, oi, 0:DCONV - 1], 0.0)

                for half in range(2):
                    ipw = ipool.tile([P, KC, DI], F32R, tag="ipw")
                    nc.sync.dma_start(
                        ipw[:], wd["in_proj_w"][:, half * DI:(half + 1) * DI]
                        .rearrange("(c p) o -> p c o", p=P).bitcast(F32R))
                    for oi in range(KI):
                        for th in range(TH):
                            ps = ps_ip.tile([P, SEG], F32, tag="ip_ps")
                            for kc in range(KC):
                                nc.tensor.matmul(
                                    ps[:], ipw[:, kc, oi * P:(oi + 1) * P],
                                    xn1T[:, kc, th * SEG:(th + 1) * SEG],
                                    start=(kc == 0), stop=(kc == KC - 1))
                            if half == 0:
                                nc.scalar.copy(
                                    xmT_pad[:, oi, DCONV - 1 + th * SEG:DCONV - 1 + (th + 1) * SEG],
                                    ps[:])
                            else:
                                sg = iwork.tile([P, SEG], F32, tag="sg_t")
                                nc.scalar.activation(sg[:], ps[:], AF.Sigmoid)
                                nc.vector.tensor_mul(
                                    sresT[:, oi, th * SEG:(th + 1) * SEG], ps[:], sg[:])

                # causal depthwise conv + silu -> xcT
                for oi in range(KI):
                    acc = iwork.tile([P, L], F32, tag="cv0")
                    nc.vector.scalar_tensor_tensor(
                        out=acc[:], in0=xmT_pad[:, oi, 0:L], scalar=conv_w_sb[:, oi, 0:1],
                        in1=conv_b_sb[:, oi:oi + 1].to_broadcast([P, L]),
                        op0=OP.mult, op1=OP.add)
                    for k in range(1, DCONV):
                        nc.vector.scalar_tensor_tensor(
                            out=acc[:], in0=xmT_pad[:, oi, k:k + L],
                            scalar=conv_w_sb[:, oi, k:k + 1], in1=acc[:],
                            op0=OP.mult, op1=OP.add)
                    for th in range(TH):
                        tsl = slice(th * SEG, (th + 1) * SEG)
                        sg2 = iwork.tile([P, SEG], F32, tag="sg2_t")
                        nc.scalar.activation(sg2[:], acc[:, tsl], AF.Sigmoid)
                        nc.vector.tensor_mul(xcT[:, oi, tsl], acc[:, tsl], sg2[:])

                # x_proj -> BCT [2*DS, L]
                xpw = ipool.tile([P, KI, 2 * DS], BF16, tag="xpw")
                nc.gpsimd.dma_start(
                    xpw[:], wd["x_proj_w"][:].rearrange("(c p) s -> p c s", p=P))
                for th in range(TH):
                    ps = ps_ip.tile([2 * DS, SEG], F32, tag="xp_ps")
                    for kc in range(KI):
                        nc.tensor.matmul(
                            ps[:], xpw[:, kc, :], xcT[:, kc, th * SEG:(th + 1) * SEG],
                            start=(kc == 0), stop=(kc == KI - 1))
                    nc.scalar.copy(BCT[:, th * SEG:(th + 1) * SEG], ps[:])
                nc.sync.dma_start(BCT_dram[:], BCT[:])

            # ---------- selective scan ----------
            hcarry = mpool.tile([P, KI, DS], F32, tag="hcarry")

            with tc.tile_pool(name="sweights", bufs=1) as swp, \
                    tc.tile_pool(name="mwork", bufs=2) as mwork, \
                    tc.tile_pool(name="pbc", bufs=1) as pbc, \
                    tc.tile_pool(name="ps_scan", bufs=1, space="PSUM") as ps_scan:
                opw = swp.tile([P, KI, DM], F32R, tag="opw")
                nc.sync.dma_start(
                    opw[:], wd["out_proj_w"][:].rearrange("(c p) o -> p c o", p=P).bitcast(F32R))
                for th in range(TH):
                    sl = slice(th * SEG, (th + 1) * SEG)
                    B_rep = pbc.tile([P, DS, SEG], BF16, tag="B_rep")
                    C_rep = pbc.tile([P, DS, SEG], BF16, tag="C_rep")
                    for n in range(DS):
                        nc.gpsimd.dma_start(
                            B_rep[:, n, :], BCT_dram[n:n + 1, sl].partition_broadcast(P))
                        nc.gpsimd.dma_start(
                            C_rep[:, n, :], BCT_dram[DS + n:DS + n + 1, sl].partition_broadcast(P))

                    psum_op = [ps_scan.tile([P, SEG], F32, tag=f"op_ps{mt}",
                                            name=f"op_ps{mt}") for mt in range(KC)]

                    for oi in range(KI):
                        dtw = mwork.tile([P, KI, P], BF16, tag="dtw")
                        nc.gpsimd.dma_start(
                            dtw[:], wd["dt_proj_w"][:, oi * P:(oi + 1) * P]
                            .rearrange("(c p) o -> p c o", p=P))
                        ps_dt = ps_scan.tile([P, SEG], F32, tag="dt_ps")
                        for kc in range(KI):
                            nc.tensor.matmul(ps_dt[:], dtw[:, kc, :], xcT[:, kc, sl],
                                             start=(kc == 0), stop=(kc == KI - 1))
                        # softplus(z) = ln(exp(z) + 1), z = psum + dt_b
                        delta = mwork.tile([P, SEG], F32, tag="delta")
                        nc.scalar.activation(delta[:], ps_dt[:], AF.Exp,
                                             bias=dt_b_sb[:, oi:oi + 1])
                        nc.scalar.activation(delta[:], delta[:], AF.Ln, bias=1.0)
                        du = mwork.tile([P, SEG], BF16, tag="du")
                        nc.vector.tensor_mul(du[:], delta[:], xcT[:, oi, sl])

                        ps_y = ps_scan.tile([P, SEG], F32, tag="y_ps")
                        for n in range(DS):
                            dA = mwork.tile([P, SEG], F32, tag="dA")
                            nc.scalar.activation(dA[:], delta[:], AF.Exp,
                                                 scale=A_rep[:, n:n + 1])
                            dBx = mwork.tile([P, SEG], F32, tag="dBx")
                            nc.vector.tensor_mul(dBx[:], B_rep[:, n, :], du[:])
                            h = mwork.tile([P, SEG], F32, tag="h")
                            init = 0.0 if th == 0 else hcarry[:, oi, n:n + 1]
                            nc.vector.tensor_tensor_scan(
                                out=h[:], data0=dA[:], data1=dBx[:], initial=init,
                                op0=OP.mult, op1=OP.add)
                            if th != TH - 1:
                                nc.gpsimd.tensor_copy(hcarry[:, oi, n:n + 1],
                                                      h[:, SEG - 1:SEG])
                            ch = mwork.tile([P, SEG], F32R, tag="ch")
                            nc.gpsimd.tensor_tensor(out=ch[:], in0=h[:],
                                                    in1=C_rep[:, n, :], op=OP.mult)
                            nc.tensor.matmul(ps_y[:], ident_r[:], ch[:],
                                             start=(n == 0), stop=(n == DS - 1))

                        t1 = mwork.tile([P, SEG], F32, tag="t1")
                        nc.vector.scalar_tensor_tensor(
                            out=t1[:], in0=xcT[:, oi, sl], scalar=D_sb[:, oi:oi + 1],
                            in1=ps_y[:], op0=OP.mult, op1=OP.add)
                        yg = mwork.tile([P, SEG], F32R, tag="yg")
                        nc.vector.tensor_mul(yg[:], t1[:], sresT[:, oi, sl])

                        for mt in range(KC):
                            nc.tensor.matmul(
                                psum_op[mt][:], opw[:, oi, mt * P:(mt + 1) * P], yg[:],
                                start=(oi == 0), stop=(oi == KI - 1))

                    for mt in range(KC):
                        nc.scalar.copy(m_outT[:, mt, sl], psum_op[mt][:])

        # mamba_out natural + x1 = x + mamba_out
        for it in range(NT):
            for jc in range(KC):
                ps = ptr.tile([P, P], F32, tag="tr_ps")
                nc.tensor.transpose(
                    ps[:], m_outT[:, jc, it * P:(it + 1) * P].bitcast(F32), ident_f[:])
                nc.scalar.copy(m_out[:, it, jc * P:(jc + 1) * P], ps[:])
            nc.vector.tensor_add(x_acc[:, it, :], x_acc[:, it, :], m_out[:, it, :])
            if debug:
                nc.sync.dma_start(
                    dbg_m[:].rearrange("(n p) d -> p n d", p=P)[:, it, :], m_out[:, it, :])

        # ================= attention =================
        with tc.tile_pool(name="attn", bufs=1) as apool:
            xn2 = apool.tile([P, NT, DM], F32, tag="xn2")
            attn_oT = apool.tile([P, KC, L], F32R, tag="attn_oT")

            with tc.tile_pool(name="qk", bufs=1) as qpool:
                QT = qpool.tile([P, KC, L], F32R, tag="QT")
                KT = qpool.tile([P, KC, L], F32R, tag="KT")
                V_sb = qpool.tile([P, NT, DM], F32R, tag="V_sb")

                with tc.tile_pool(name="wqkv", bufs=1) as wpool, \
                        tc.tile_pool(name="ps_qkv", bufs=2, space="PSUM") as ps_qkv:
                    xn2T = wpool.tile([P, KC, L], F32R, tag="xn2T")
                    for it in range(NT):
                        cs = center_scale(x_acc[:, it, :])
                        nc.vector.tensor_mul(xn2[:, it, :], cs[:], g2_rep[:])
                        nc.vector.tensor_add(xn2[:, it, :], xn2[:, it, :], b2_rep[:])
                        transpose_apply(cs[:], it, g2_sb, b2n_sb, xn2T)

                    wq_sb = wpool.tile([P, KC, DM], F32R, tag="wq_sb")
                    nc.sync.dma_start(
                        wq_sb[:], wd["wq_w"][:].rearrange("(c p) o -> p c o", p=P).bitcast(F32R))
                    wk_sb = wpool.tile([P, KC, DM], F32R, tag="wk_sb")
                    nc.sync.dma_start(
                        wk_sb[:], wd["wk_w"][:].rearrange("(c p) o -> p c o", p=P).bitcast(F32R))
                    wv_sb = wpool.tile([P, KC, DM], F32R, tag="wv_sb")
                    nc.sync.dma_start(
                        wv_sb[:], wd["wv_w"][:].rearrange("(c p) o -> p c o", p=P).bitcast(F32R))

                    for hg in range(KC):
                        for th in range(TH):
                            for (w_sb, bias_sb, dstT) in (
                                    (wq_sb, wqb_sb, QT), (wk_sb, wkb_sb, KT)):
                                ps = ps_qkv.tile([P, SEG], F32, tag="qk_ps")
                                for kc in range(KC):
                                    nc.tensor.matmul(
                                        ps[:], w_sb[:, kc, hg * P:(hg + 1) * P],
                                        xn2T[:, kc, th * SEG:(th + 1) * SEG],
                                        start=(kc == 0), stop=(kc == KC - 1))
                                nc.scalar.activation(
                                    dstT[:, hg, th * SEG:(th + 1) * SEG], ps[:],
                                    AF.Identity, scale=1.0, bias=bias_sb[:, hg:hg + 1])

                    for it in range(NT):
                        ps = ps_qkv.tile([P, DM], F32, tag="v_ps")
                        for kc in range(KC):
                            nc.tensor.matmul(
                                ps[:], xn2T[:, kc, it * P:(it + 1) * P], wv_sb[:, kc, :],
                                start=(kc == 0), stop=False)
                        nc.tensor.matmul(ps[:], ones_r[:], wvb_row[:],
                                         start=False, stop=True)
                        nc.scalar.copy(V_sb[:, it, :], ps[:])

                # per-head attention
                inv_sqrt = 1.0 / float(np.sqrt(DK))
                with tc.tile_pool(name="pet", bufs=3) as pet, \
                        tc.tile_pool(name="pew", bufs=2) as pew, \
                        tc.tile_pool(name="ps_av", bufs=1, space="PSUM") as ps_avp, \
                        tc.tile_pool(name="ps_hd", bufs=2, space="PSUM") as ps_hd:
                    for h in range(H):
                        hg, hh = h // 2, h % 2
                        qsl = slice(hh * DK, (hh + 1) * DK)

                        # natural scores -> E, rowsums -> W -> DRAM; recips -> DRAM
                        for iq in range(NT):
                            e_t = pew.tile([P, L], F32, tag="e_t")
                            rs = pstat.tile([P, TH], F32, tag="rs")
                            for kh in range(TH):
                                ps = ps_hd.tile([P, SEG], F32, tag="s_ps")
                                nc.tensor.matmul(
                                    ps[:], QT[qsl, hg, iq * P:(iq + 1) * P],
                                    KT[qsl, hg, kh * SEG:(kh + 1) * SEG],
                                    start=True, stop=True)
                                nc.scalar.activation(
                                    e_t[:, kh * SEG:(kh + 1) * SEG], ps[:], AF.Exp,
                                    scale=inv_sqrt, accum_out=rs[:, kh:kh + 1])
                            rsum = pstat.tile([P, 1], F32, tag="rsum")
                            if TH == 1:
                                nc.vector.reciprocal(rsum[:], rs[:, 0:1])
                            else:
                                nc.vector.tensor_add(rsum[:], rs[:, 0:1], rs[:, 1:2])
                                for kh in range(2, TH):
                                    nc.vector.tensor_add(rsum[:], rsum[:], rs[:, kh:kh + 1])
                                nc.vector.reciprocal(rsum[:], rsum[:])
                            nc.sync.dma_start(
                                recip_dram[h, iq * P:(iq + 1) * P, :], rsum[:])
                            nc.scalar.activation(e_t[:], e_t[:], AF.Identity,
                                                 scale=rsum[:])
                            nc.sync.dma_start(
                                attn_w_out[h, iq * P:(iq + 1) * P, :], e_t[:])

                        # scores^T -> exp -> ET, attn@V accumulated per th
                        rec_rep = pew.tile([P, L], F32, tag="rec_rep")
                        nc.sync.dma_start(
                            rec_rep[:],
                            recip_dram[h, :, 0][None, :].partition_broadcast(P))
                        ps_av = [ps_avp.tile([DK, SEG], F32, tag=f"av_ps{th}",
                                             name=f"av_ps{th}") for th in range(TH)]
                        for ik in range(NT):
                            et = pet.tile([P, L], F32R, tag="et")
                            for th in range(TH):
                                ps = ps_hd.tile([P, SEG], F32, tag="sT_ps")
                                nc.tensor.matmul(
                                    ps[:], KT[qsl, hg, ik * P:(ik + 1) * P],
                                    QT[qsl, hg, th * SEG:(th + 1) * SEG],
                                    start=True, stop=True)
                                nc.scalar.activation(
                                    et[:, th * SEG:(th + 1) * SEG], ps[:], AF.Exp,
                                    scale=inv_sqrt)
                            for th in range(TH):
                                nc.tensor.matmul(
                                    ps_av[th][:], V_sb[:, ik, h * DK:(h + 1) * DK],
                                    et[:, th * SEG:(th + 1) * SEG],
                                    start=(ik == 0), stop=(ik == NT - 1))
                        for th in range(TH):
                            tsl = slice(th * SEG, (th + 1) * SEG)
                            if hh == 0:
                                nc.vector.tensor_mul(
                                    attn_oT[0:DK, hg, tsl], ps_av[th][:],
                                    rec_rep[0:DK, tsl])
                            else:
                                t_av = pew.tile([DK, SEG], F32R, tag="t_av")
                                nc.vector.tensor_mul(
                                    t_av[:], ps_av[th][:], rec_rep[0:DK, tsl])
                                nc.sync.dma_start(attn_oT[DK:P, hg, tsl], t_av[:])

            # wo + residual + attn LN + gate + fused combine
            with tc.tile_pool(name="awork", bufs=2) as awork, \
                    tc.tile_pool(name="apost", bufs=1) as apost, \
                    tc.tile_pool(name="ps_wo", bufs=2, space="PSUM") as ps_wo:
                a_out = apost.tile([P, NT, DM], F32, tag="a_out")
                a_outT = apost.tile([P, KC, L], F32R, tag="a_outT")
                wo_sb = apost.tile([P, KC, DM], F32R, tag="wo_sb")
                nc.sync.dma_start(
                    wo_sb[:], wd["wo_w"][:].rearrange("(c p) o -> p c o", p=P).bitcast(F32R))
                for it in range(NT):
                    ps = ps_wo.tile([P, DM], F32, tag="wo_ps")
                    for kc in range(KC):
                        nc.tensor.matmul(
                            ps[:], attn_oT[:, kc, it * P:(it + 1) * P], wo_sb[:, kc, :],
                            start=(kc == 0), stop=False)
                    nc.tensor.matmul(ps[:], ones_r[:], wob_row[:], start=False, stop=True)
                    r2 = awork.tile([P, DM], F32, tag="r2")
                    nc.vector.tensor_add(r2[:], ps[:], xn2[:, it, :])
                    cs = center_scale(r2[:])
                    nc.vector.tensor_mul(a_out[:, it, :], cs[:], ga_rep[:])
                    nc.vector.tensor_add(a_out[:, it, :], a_out[:, it, :], ba_rep[:])
                    transpose_apply(cs[:], it, ga_sb, ba_sb, a_outT)
                    if debug:
                        nc.sync.dma_start(
                            dbg_a[:].rearrange("(n p) d -> p n d", p=P)[:, it, :],
                            a_out[:, it, :])

                gw_sb = apost.tile([P, 2 * KC, DM], F32R, tag="gw_sb")
                nc.sync.dma_start(
                    gw_sb[:], wd["gate_w"][:].rearrange("(c p) o -> p c o", p=P).bitcast(F32R))
                for it in range(NT):
                    ps = ps_wo.tile([P, DM], F32, tag="g_ps")
                    for kc in range(KC):
                        nc.tensor.matmul(
                            ps[:], m_outT[:, kc, it * P:(it + 1) * P], gw_sb[:, kc, :],
                            start=(kc == 0), stop=False)
                    for kc in range(KC):
                        nc.tensor.matmul(
                            ps[:], a_outT[:, kc, it * P:(it + 1) * P],
                            gw_sb[:, KC + kc, :], start=False, stop=False)
                    nc.tensor.matmul(ps[:], ones_r[:], gateb_row[:], start=False, stop=True)
                    g_t = awork.tile([P, DM], F32, tag="g_t")
                    nc.scalar.activation(g_t[:], ps[:], AF.Sigmoid)
                    if debug:
                        nc.sync.dma_start(
                            dbg_g[:].rearrange("(n p) d -> p n d", p=P)[:, it, :], g_t[:])
                    # x3 = x2 + fused = x1 + 2*a + g*(m - a)
                    t1 = awork.tile([P, DM], F32, tag="f_t1")
                    nc.vector.tensor_tensor(out=t1[:], in0=m_out[:, it, :],
                                            in1=a_out[:, it, :], op=OP.subtract)
                    t2 = awork.tile([P, DM], F32, tag="f_t2")
                    nc.vector.tensor_mul(t2[:], g_t[:], t1[:])
                    nc.vector.scalar_tensor_tensor(
                        out=t2[:], in0=a_out[:, it, :], scalar=2.0, in1=t2[:],
                        op0=OP.mult, op1=OP.add)
                    nc.vector.tensor_add(x_acc[:, it, :], x_acc[:, it, :], t2[:])
                    if debug:
                        nc.sync.dma_start(
                            dbg_x3[:].rearrange("(n p) d -> p n d", p=P)[:, it, :],
                            x_acc[:, it, :])

        mid_cm.__exit__(None, None, None)

        # ================= FFN =================
        with tc.tile_pool(name="ffn", bufs=1) as fpool, \
                tc.tile_pool(name="fwork", bufs=2) as fwork, \
                tc.tile_pool(name="ps_ffn", bufs=2, space="PSUM") as ps_ffn:
            xn3T = fpool.tile([P, KC, L], BF16, tag="xn3T")
            for it in range(NT):
                cs = center_scale(x_acc[:, it, :])
                for jc in range(KC):
                    ps = ptr.tile([P, P], F32, tag="tr_ps")
                    nc.tensor.transpose(ps[:], cs[:, jc * P:(jc + 1) * P], ident_f[:])
                    nc.scalar.activation(
                        xn3T[:, jc, it * P:(it + 1) * P], ps[:], AF.Identity,
                        scale=g3_sb[:, jc:jc + 1], bias=b3n_sb[:, jc:jc + 1])

            f1_sb = fpool.tile([P, KC, DFF], BF16, tag="f1_sb")
            nc.gpsimd.dma_start(
                f1_sb[:], wd["ffn_w1"][:].rearrange("(c p) o -> p c o", p=P))
            h1T = fpool.tile([P, KF, L], BF16, tag="h1T")
            for ff in range(KF):
                for th in range(TH):
                    ps = ps_ffn.tile([P, SEG], F32, tag="f1_ps")
                    for kc in range(KC):
                        nc.tensor.matmul(
                            ps[:], f1_sb[:, kc, ff * P:(ff + 1) * P],
                            xn3T[:, kc, th * SEG:(th + 1) * SEG],
                            start=(kc == 0), stop=(kc == KC - 1))
                    # gelu_tanh(x) = 0.5*x*(1+tanh(c1*x + c2*x^3)), x = psum + b1
                    x_t = fwork.tile([P, SEG], F32, tag="x_t")
                    nc.scalar.activation(x_t[:], ps[:], AF.Identity,
                                         bias=fb1_sb[:, ff:ff + 1])
                    s_t = fwork.tile([P, SEG], F32, tag="s_t")
                    nc.scalar.activation(s_t[:], x_t[:], AF.Square)
                    p_t = fwork.tile([P, SEG], F32, tag="p_t")
                    nc.vector.tensor_scalar(
                        out=p_t[:], in0=s_t[:], scalar1=0.044715 * 0.7978845608028654,
                        scalar2=0.7978845608028654, op0=OP.mult, op1=OP.add)
                    nc.vector.tensor_mul(p_t[:], p_t[:], x_t[:])
                    th_t = fwork.tile([P, SEG], F32, tag="th_t")
                    nc.scalar.activation(th_t[:], p_t[:], AF.Tanh)
                    q_t = fwork.tile([P, SEG], F32, tag="q_t")
                    nc.vector.tensor_scalar(
                        out=q_t[:], in0=th_t[:], scalar1=1.0, scalar2=0.5,
                        op0=OP.add, op1=OP.mult)
                    nc.vector.tensor_mul(
                        h1T[:, ff, th * SEG:(th + 1) * SEG], q_t[:], x_t[:])
                    if debug:
                        h1f = fwork.tile([P, SEG], F32, tag="h1f")
                        nc.vector.tensor_mul(h1f[:], q_t[:], x_t[:])
                        nc.sync.dma_start(
                            dbg_h1[:].rearrange("(f p) t -> p f t", p=P)[:, ff, th * SEG:(th + 1) * SEG],
                            h1f[:])

            f2_sb = fpool.tile([P, KF, DM], BF16, tag="f2_sb")
            nc.gpsimd.dma_start(
                f2_sb[:], wd["ffn_w2"][:].rearrange("(c p) o -> p c o", p=P))
            for it in range(NT):
                ps = ps_ffn.tile([P, DM], F32, tag="f2_ps")
                for kc in range(KF):
                    nc.tensor.matmul(
                        ps[:], h1T[:, kc, it * P:(it + 1) * P], f2_sb[:, kc, :],
                        start=(kc == 0), stop=False)
                nc.tensor.matmul(ps[:], ones_r[:], fb2_row[:], start=False, stop=True)
                o_t = fwork.tile([P, DM], F32, tag="o_t")
                nc.vector.tensor_add(o_t[:], ps[:], x_acc[:, it, :])
                nc.sync.dma_start(
                    out_x[:].rearrange("(n p) d -> p n d", p=P)[:, it, :], o_t[:])

    nc.compile()
    return nc


_CACHE = {}


def _get_program(L):
    if L not in _CACHE:
        _CACHE[L] = build_program(L)
    return _CACHE[L]


def kernel(**inputs):
    from concourse.bass_utils import run_bass_kernel_spmd

    x = np.ascontiguousarray(inputs["x"], dtype=np.float32)
    Bx, L, _ = x.shape
    nc = _get_program(L)

    weights = {}
    for name in ["in_proj_w", "conv_w", "conv_b", "x_proj_w", "dt_proj_w", "dt_proj_b",
                 "A_log", "D", "out_proj_w", "wq_w", "wq_b", "wk_w", "wk_b", "wv_w",
                 "wv_b", "wo_w", "wo_b", "attn_ln_g", "attn_ln_b", "ffn_w1", "ffn_b1",
                 "ffn_w2", "ffn_b2", "norm1_g", "norm1_b", "norm2_g", "norm2_b",
                 "norm3_g", "norm3_b", "gate_w", "gate_b"]:
        weights[name] = np.ascontiguousarray(inputs[name], dtype=np.float32)
    weights["ident"] = np.eye(P, dtype=np.float32)
    weights["ones_row"] = np.ones((1, P), dtype=np.float32)

    in_maps = [{"x_in": np.ascontiguousarray(x[b]), **weights} for b in range(Bx)]
    res = run_bass_kernel_spmd(nc, in_maps, core_ids=list(range(Bx)))
    out = np.stack([r["out_x"] for r in res.results], axis=0)
    attn_w = np.stack([r["attn_w_out"] for r in res.results], axis=0)
    return (out, attn_w)


if __name__ == "__main__":
    build_program(256)
    print("built OK")


# revision 20
# speedup vs baseline: 1.0473x; 1.0473x over previous
"""EnhancedMambaFormerBlock Trainium2 kernel.

Data-parallel over batch: 8 batch elements -> 8 NeuronCores, no collectives.
Per core the full block (LN -> Mamba(conv + selective scan) -> LN -> attention
-> gated fusion -> LN -> FFN) runs with a transposed-layout compute chain:

 - Matmuls run in fp32r (full PE speed at N>=256, ~1e-4 rounding) or bf16
   where precision allows (dt_proj / x_proj / ffn).
 - The selective scan uses the DVE `tensor_tensor_scan` instruction
   (state = dA * state + dBx along time), one scan per
   (state n, 128-channel block, 512-time segment); decay factors come from
   ScalarE exp(A_n * delta); the C-contraction over n uses GpSimd multiplies
   + PE identity-matmul accumulation in PSUM.
 - Attention scores are computed in BOTH layouts ([tq,tk] for softmax /
   attn_w output, [tk,tq] for attn@V); exp + row sums via ScalarE accum_out.

Self-contained: hardcodes all shapes from the problem spec.
"""
import sys
from contextlib import ExitStack

import numpy as np

sys.path.insert(0, "/opt/trn_rl_repo")

import concourse.tile as tile  # noqa: E402
from concourse import bacc, mybir  # noqa: E402

F32 = mybir.dt.float32
F32R = mybir.dt.float32r
BF16 = mybir.dt.bfloat16
AF = mybir.ActivationFunctionType
OP = mybir.AluOpType

DM = 512
DS = 16
DCONV = 4
DI = 1024
H = 8
DK = 64
DFF = 2048
EPS = 1e-5
P = 128

KC = DM // P   # 4  dmodel chunks
KI = DI // P   # 8  d_inner chunks
KF = DFF // P  # 16 dff chunks


def build_program(L=1024, debug=False):
    NT = L // P
    SEG = min(512, L)
    TH = L // SEG

    nc = bacc.Bacc("TRN2", target_bir_lowering=False, debug=False)

    x_in = nc.dram_tensor("x_in", [L, DM], F32, kind="ExternalInput")
    out_x = nc.dram_tensor("out_x", [L, DM], F32, kind="ExternalOutput")
    attn_w_out = nc.dram_tensor("attn_w_out", [H, L, L], F32, kind="ExternalOutput")
    if debug:
        dbg_m = nc.dram_tensor("dbg_m", [L, DM], F32, kind="ExternalOutput")
        dbg_a = nc.dram_tensor("dbg_a", [L, DM], F32, kind="ExternalOutput")
        dbg_g = nc.dram_tensor("dbg_g", [L, DM], F32, kind="ExternalOutput")
        dbg_x3 = nc.dram_tensor("dbg_x3", [L, DM], F32, kind="ExternalOutput")
        dbg_h1 = nc.dram_tensor("dbg_h1", [DFF, L], F32, kind="ExternalOutput")

    wd = {}
    for name, shape in [
        ("in_proj_w", [DM, 2 * DI]), ("conv_w", [DI, DCONV]), ("conv_b", [DI]),
        ("x_proj_w", [DI, 2 * DS]), ("dt_proj_w", [DI, DI]), ("dt_proj_b", [DI]),
        ("A_log", [DS]), ("D", [DI]), ("out_proj_w", [DI, DM]),
        ("wq_w", [DM, DM]), ("wq_b", [DM]), ("wk_w", [DM, DM]), ("wk_b", [DM]),
        ("wv_w", [DM, DM]), ("wv_b", [DM]), ("wo_w", [DM, DM]), ("wo_b", [DM]),
        ("attn_ln_g", [DM]), ("attn_ln_b", [DM]),
        ("ffn_w1", [DM, DFF]), ("ffn_b1", [DFF]), ("ffn_w2", [DFF, DM]), ("ffn_b2", [DM]),
        ("norm1_g", [DM]), ("norm1_b", [DM]), ("norm2_g", [DM]), ("norm2_b", [DM]),
        ("norm3_g", [DM]), ("norm3_b", [DM]),
        ("gate_w", [2 * DM, DM]), ("gate_b", [DM]),
        ("ident", [P, P]), ("ones_row", [1, P]),
    ]:
        wd[name] = nc.dram_tensor(name, shape, F32, kind="ExternalInput")

    with tile.TileContext(nc) as tc, ExitStack() as top:
        persist = top.enter_context(tc.tile_pool(name="persist", bufs=1))
        dscratch = top.enter_context(tc.tile_pool(name="dscratch", bufs=1, space="DRAM"))
        pwork = top.enter_context(tc.tile_pool(name="pwork", bufs=2))
        pstat = top.enter_context(tc.tile_pool(name="pstat", bufs=4))
        ptr = top.enter_context(tc.tile_pool(name="ptr", bufs=2, space="PSUM"))

        # ---------------- consts ----------------
        ident_f = persist.tile([P, P], F32, tag="ident_f")
        nc.sync.dma_start(ident_f[:], wd["ident"][:])
        ident_r = persist.tile([P, P], F32R, tag="ident_r")
        nc.sync.dma_start(ident_r[:], wd["ident"][:].bitcast(F32R))
        ones_r = persist.tile([1, P], F32R, tag="ones_r")
        nc.sync.dma_start(ones_r[:], wd["ones_row"][:].bitcast(F32R))
        ident_b = persist.tile([P, P], BF16, tag="ident_b")
        nc.gpsimd.dma_start(ident_b[:], wd["ident"][:])
        eps_t = persist.tile([P, 1], F32, tag="eps")
        nc.vector.memset(eps_t[:], EPS)

        alog_rep = persist.tile([P, DS], F32, tag="alog_rep")
        nc.sync.dma_start(alog_rep[:], wd["A_log"][None, :].partition_broadcast(P))
        A_rep = persist.tile([P, DS], F32, tag="A_rep")
        nc.scalar.activation(A_rep[:], alog_rep[:], AF.Exp)
        nc.vector.tensor_scalar_mul(A_rep[:], A_rep[:], -1.0)

        def ppart(name, cols, tag):
            t = persist.tile([P, cols], F32, tag=tag)
            nc.sync.dma_start(t[:], wd[name][:].rearrange("(o p) -> p o", p=P))
            return t

        conv_b_sb = ppart("conv_b", KI, "conv_b")
        dt_b_sb = ppart("dt_proj_b", KI, "dt_b")
        D_sb = ppart("D", KI, "D_sb")
        g1_sb = ppart("norm1_g", KC, "g1")
        b1n_sb = ppart("norm1_b", KC, "b1n")
        g2_sb = ppart("norm2_g", KC, "g2")
        b2n_sb = ppart("norm2_b", KC, "b2n")
        g3_sb = ppart("norm3_g", KC, "g3")
        b3n_sb = ppart("norm3_b", KC, "b3n")
        ga_sb = ppart("attn_ln_g", KC, "ga")
        ba_sb = ppart("attn_ln_b", KC, "ba")
        wqb_sb = ppart("wq_b", KC, "wqb")
        wkb_sb = ppart("wk_b", KC, "wkb")
        fb1_sb = ppart("ffn_b1", KF, "fb1")
        conv_w_sb = persist.tile([P, KI, DCONV], F32, tag="conv_w")
        nc.sync.dma_start(conv_w_sb[:], wd["conv_w"][:].rearrange("(o p) k -> p o k", p=P))

        def vrep(name, tag):
            t = persist.tile([P, DM], F32, tag=tag)
            nc.sync.dma_start(t[:], wd[name][None, :].partition_broadcast(P))
            return t

        g2_rep = vrep("norm2_g", "g2rep")
        b2_rep = vrep("norm2_b", "b2rep")
        ga_rep = vrep("attn_ln_g", "garep")
        ba_rep = vrep("attn_ln_b", "barep")

        def brow(name, n, tag):
            t = persist.tile([1, n], F32R, tag=tag)
            nc.sync.dma_start(t[:], wd[name][None, :].bitcast(F32R))
            return t

        wvb_row = brow("wv_b", DM, "wvb_row")
        wob_row = brow("wo_b", DM, "wob_row")
        gateb_row = brow("gate_b", DM, "gateb_row")
        fb2_row = brow("ffn_b2", DM, "fb2_row")

        recip_dram = dscratch.tile([H, L, 1], F32, tag="recip_dram")
        x_acc = persist.tile([P, NT, DM], F32, tag="x_acc")
        nc.sync.dma_start(x_acc[:], x_in[:].rearrange("(n p) d -> p n d", p=P))



        # ---------------- helpers ----------------
        def ln_stats(xt):
            st = pstat.tile([P, 6], F32, tag="st6")
            nc.vector.bn_stats(st[:], xt)
            mv = pstat.tile([P, 2], F32, tag="mv2")
            nc.vector.bn_aggr(mv[:], st[:])
            rstd = pstat.tile([P, 1], F32, tag="rstd")
            nc.scalar.activation(rstd[:], mv[:, 1:2], AF.Sqrt, bias=eps_t[:])
            nc.vector.reciprocal(rstd[:], rstd[:])
            return mv[:, 0:1], rstd

        def center_scale(src):
            mu, rstd = ln_stats(src)
            cs = pwork.tile([P, DM], F32, tag="cs_t")
            nc.vector.tensor_scalar(
                out=cs[:], in0=src, scalar1=mu, scalar2=rstd,
                op0=OP.subtract, op1=OP.mult)
            return cs

        def transpose_apply(cs_t, it, g_col, b_col, dstT):
            for jc in range(KC):
                ps = ptr.tile([P, P], F32, tag="tr_ps")
                nc.tensor.transpose(ps[:], cs_t[:, jc * P:(jc + 1) * P], ident_f[:])
                nc.scalar.activation(
                    dstT[:, jc, it * P:(it + 1) * P], ps[:], AF.Identity,
                    scale=g_col[:, jc:jc + 1], bias=b_col[:, jc:jc + 1])

        # ================= mamba =================
        mid_cm = tc.tile_pool(name="mid", bufs=1)
        mid = mid_cm.__enter__()
        m_outT = mid.tile([P, KC, L], F32R, tag="m_outT")
        m_out = mid.tile([P, NT, DM], F32, tag="m_out")
        with tc.tile_pool(name="mamba", bufs=1) as mpool:
            xcT = mpool.tile([P, KI, L], BF16, tag="xcT")
            BCT_dram = dscratch.tile([2 * DS, L], F32, tag="BCT_dram")
            sresT = mpool.tile([P, KI, L], BF16, tag="sresT")
            BCT = mpool.tile([2 * DS, L], F32, tag="BCT")

            with tc.tile_pool(name="inproj", bufs=1) as ipool, \
                    tc.tile_pool(name="iwork", bufs=2) as iwork, \
                    tc.tile_pool(name="ps_ip", bufs=2, space="PSUM") as ps_ip:
                xn1T = ipool.tile([P, KC, L], F32R, tag="xn1T")
                for it in range(NT):
                    cs = center_scale(x_acc[:, it, :])
                    transpose_apply(cs[:], it, g1_sb, b1n_sb, xn1T)

                xmT_pad = ipool.tile([P, KI, L + DCONV - 1], F32, tag="xmT_pad")
                for oi in range(KI):
                    nc.vector.memset(xmT_pad[:, oi, 0:DCONV - 1], 0.0)

                for half in range(2):
                    ipw = ipool.tile([P, KC, DI], F32R, tag="ipw")
                    nc.sync.dma_start(
                        ipw[:], wd["in_proj_w"][:, half * DI:(half + 1) * DI]
                        .rearrange("(c p) o -> p c o", p=P).bitcast(F32R))
                    for oi in range(KI):
                        for th in range(TH):
                            ps = ps_ip.tile([P, SEG], F32, tag="ip_ps")
                            for kc in range(KC):
                                nc.tensor.matmul(
                                    ps[:], ipw[:, kc, oi * P:(oi + 1) * P],
                                    xn1T[:, kc, th * SEG:(th + 1) * SEG],
                                    start=(kc == 0), stop=(kc == KC - 1))
                            if half == 0:
                                nc.scalar.copy(
                                    xmT_pad[:, oi, DCONV - 1 + th * SEG:DCONV - 1 + (th + 1) * SEG],
                                    ps[:])
                            else:
                                sg = iwork.tile([P, SEG], F32, tag="sg_t")
                                nc.scalar.activation(sg[:], ps[:], AF.Sigmoid)
                                nc.vector.tensor_mul(
                                    sresT[:, oi, th * SEG:(th + 1) * SEG], ps[:], sg[:])

                # causal depthwise conv + silu -> xcT
                for oi in range(KI):
                    acc = iwork.tile([P, L], F32, tag="cv0")
                    nc.vector.scalar_tensor_tensor(
                        out=acc[:], in0=xmT_pad[:, oi, 0:L], scalar=conv_w_sb[:, oi, 0:1],
                        in1=conv_b_sb[:, oi:oi + 1].to_broadcast([P, L]),
                        op0=OP.mult, op1=OP.add)
                    for k in range(1, DCONV):
                        nc.vector.scalar_tensor_tensor(
                            out=acc[:], in0=xmT_pad[:, oi, k:k + L],
                            scalar=conv_w_sb[:, oi, k:k + 1], in1=acc[:],
                            op0=OP.mult, op1=OP.add)
                    for th in range(TH):
                        tsl = slice(th * SEG, (th + 1) * SEG)
                        sg2 = iwork.tile([P, SEG], F32, tag="sg2_t")
                        nc.scalar.activation(sg2[:], acc[:, tsl], AF.Sigmoid)
                        nc.vector.tensor_mul(xcT[:, oi, tsl], acc[:, tsl], sg2[:])

                # x_proj -> BCT [2*DS, L]
                xpw = ipool.tile([P, KI, 2 * DS], BF16, tag="xpw")
                nc.gpsimd.dma_start(
                    xpw[:], wd["x_proj_w"][:].rearrange("(c p) s -> p c s", p=P))
                for th in range(TH):
                    ps = ps_ip.tile([2 * DS, SEG], F32, tag="xp_ps")
                    for kc in range(KI):
                        nc.tensor.matmul(
                            ps[:], xpw[:, kc, :], xcT[:, kc, th * SEG:(th + 1) * SEG],
                            start=(kc == 0), stop=(kc == KI - 1))
                    nc.scalar.copy(BCT[:, th * SEG:(th + 1) * SEG], ps[:])
                nc.sync.dma_start(BCT_dram[:], BCT[:])

            # ---------- selective scan ----------
            hcarry = mpool.tile([P, KI, DS], F32, tag="hcarry")

            with tc.tile_pool(name="sweights", bufs=1) as swp, \
                    tc.tile_pool(name="mwork", bufs=2) as mwork, \
                    tc.tile_pool(name="shot", bufs=3) as shot, \
                    tc.tile_pool(name="pbc", bufs=1) as pbc, \
                    tc.tile_pool(name="ps_scan", bufs=1, space="PSUM") as ps_scan:
                opw = swp.tile([P, KI, DM], F32R, tag="opw")
                nc.sync.dma_start(
                    opw[:], wd["out_proj_w"][:].rearrange("(c p) o -> p c o", p=P).bitcast(F32R))
                for th in range(TH):
                    sl = slice(th * SEG, (th + 1) * SEG)
                    B_rep = pbc.tile([P, DS, SEG], BF16, tag="B_rep")
                    C_rep = pbc.tile([P, DS, SEG], BF16, tag="C_rep")
                    for n in range(DS):
                        nc.gpsimd.dma_start(
                            B_rep[:, n, :], BCT_dram[n:n + 1, sl].partition_broadcast(P))
                        nc.gpsimd.dma_start(
                            C_rep[:, n, :], BCT_dram[DS + n:DS + n + 1, sl].partition_broadcast(P))

                    psum_op = [ps_scan.tile([P, SEG], F32, tag=f"op_ps{mt}",
                                            name=f"op_ps{mt}") for mt in range(KC)]

                    for oi in range(KI):
                        dtw = mwork.tile([P, KI, P], BF16, tag="dtw")
                        nc.gpsimd.dma_start(
                            dtw[:], wd["dt_proj_w"][:, oi * P:(oi + 1) * P]
                            .rearrange("(c p) o -> p c o", p=P))
                        ps_dt = ps_scan.tile([P, SEG], F32, tag="dt_ps")
                        for kc in range(KI):
                            nc.tensor.matmul(ps_dt[:], dtw[:, kc, :], xcT[:, kc, sl],
                                             start=(kc == 0), stop=(kc == KI - 1))
                        # softplus(z) = ln(exp(z) + 1), z = psum + dt_b
                        delta = mwork.tile([P, SEG], F32, tag="delta")
                        nc.scalar.activation(delta[:], ps_dt[:], AF.Exp,
                                             bias=dt_b_sb[:, oi:oi + 1])
                        nc.scalar.activation(delta[:], delta[:], AF.Ln, bias=1.0)
                        du = mwork.tile([P, SEG], BF16, tag="du")
                        nc.vector.tensor_mul(du[:], delta[:], xcT[:, oi, sl])

                        ps_y = ps_scan.tile([P, SEG], F32, tag="y_ps")
                        for n in range(DS):
                            dA = shot.tile([P, SEG], BF16, tag="dA")
                            nc.scalar.activation(dA[:], delta[:], AF.Exp,
                                                 scale=A_rep[:, n:n + 1])
                            dBx = shot.tile([P, SEG], BF16, tag="dBx")
                            nc.vector.tensor_mul(dBx[:], B_rep[:, n, :], du[:])
                            h = shot.tile([P, SEG], BF16, tag="h")
                            init = 0.0 if th == 0 else hcarry[:, oi, n:n + 1]
                            nc.vector.tensor_tensor_scan(
                                out=h[:], data0=dA[:], data1=dBx[:], initial=init,
                                op0=OP.mult, op1=OP.add)
                            if th != TH - 1:
                                nc.gpsimd.tensor_copy(hcarry[:, oi, n:n + 1],
                                                      h[:, SEG - 1:SEG])
                            ch = shot.tile([P, SEG], BF16, tag="ch")
                            if n % 3 == 0:
                                nc.vector.tensor_tensor(out=ch[:], in0=h[:],
                                                        in1=C_rep[:, n, :], op=OP.mult)
                            else:
                                nc.gpsimd.tensor_tensor(out=ch[:], in0=h[:],
                                                        in1=C_rep[:, n, :], op=OP.mult)
                            nc.tensor.matmul(ps_y[:], ident_b[:], ch[:],
                                             start=(n == 0), stop=(n == DS - 1))

                        t1 = mwork.tile([P, SEG], F32, tag="t1")
                        nc.vector.scalar_tensor_tensor(
                            out=t1[:], in0=xcT[:, oi, sl], scalar=D_sb[:, oi:oi + 1],
                            in1=ps_y[:], op0=OP.mult, op1=OP.add)
                        yg = mwork.tile([P, SEG], F32R, tag="yg")
                        nc.vector.tensor_mul(yg[:], t1[:], sresT[:, oi, sl])

                        for mt in range(KC):
                            nc.tensor.matmul(
                                psum_op[mt][:], opw[:, oi, mt * P:(mt + 1) * P], yg[:],
                                start=(oi == 0), stop=(oi == KI - 1))

                    for mt in range(KC):
                        nc.scalar.copy(m_outT[:, mt, sl], psum_op[mt][:])

        # mamba_out natural + x1 = x + mamba_out
        for it in range(NT):
            for jc in range(KC):
                ps = ptr.tile([P, P], F32, tag="tr_ps")
                nc.tensor.transpose(
                    ps[:], m_outT[:, jc, it * P:(it + 1) * P].bitcast(F32), ident_f[:])
                nc.scalar.copy(m_out[:, it, jc * P:(jc + 1) * P], ps[:])
            nc.vector.tensor_add(x_acc[:, it, :], x_acc[:, it, :], m_out[:, it, :])
            if debug:
                nc.sync.dma_start(
                    dbg_m[:].rearrange("(n p) d -> p n d", p=P)[:, it, :], m_out[:, it, :])

        # ================= attention =================
        with tc.tile_pool(name="attn", bufs=1) as apool:
            xn2 = apool.tile([P, NT, DM], F32, tag="xn2")
            attn_oT = apool.tile([P, KC, L], F32R, tag="attn_oT")

            with tc.tile_pool(name="qk", bufs=1) as qpool:
                QT = qpool.tile([P, KC, L], F32R, tag="QT")
                KT = qpool.tile([P, KC, L], F32R, tag="KT")
                V_sb = qpool.tile([P, NT, DM], F32R, tag="V_sb")

                with tc.tile_pool(name="wqkv", bufs=1) as wpool, \
                        tc.tile_pool(name="ps_qkv", bufs=2, space="PSUM") as ps_qkv:
                    xn2T = wpool.tile([P, KC, L], F32R, tag="xn2T")
                    for it in range(NT):
                        cs = center_scale(x_acc[:, it, :])
                        nc.vector.tensor_mul(xn2[:, it, :], cs[:], g2_rep[:])
                        nc.vector.tensor_add(xn2[:, it, :], xn2[:, it, :], b2_rep[:])
                        transpose_apply(cs[:], it, g2_sb, b2n_sb, xn2T)

                    wq_sb = wpool.tile([P, KC, DM], F32R, tag="wq_sb")
                    nc.sync.dma_start(
                        wq_sb[:], wd["wq_w"][:].rearrange("(c p) o -> p c o", p=P).bitcast(F32R))
                    wk_sb = wpool.tile([P, KC, DM], F32R, tag="wk_sb")
                    nc.sync.dma_start(
                        wk_sb[:], wd["wk_w"][:].rearrange("(c p) o -> p c o", p=P).bitcast(F32R))
                    wv_sb = wpool.tile([P, KC, DM], F32R, tag="wv_sb")
                    nc.sync.dma_start(
                        wv_sb[:], wd["wv_w"][:].rearrange("(c p) o -> p c o", p=P).bitcast(F32R))

                    for hg in range(KC):
                        for th in range(TH):
                            for (w_sb, bias_sb, dstT) in (
                                    (wq_sb, wqb_sb, QT), (wk_sb, wkb_sb, KT)):
                                ps = ps_qkv.tile([P, SEG], F32, tag="qk_ps")
                                for kc in range(KC):
                                    nc.tensor.matmul(
                                        ps[:], w_sb[:, kc, hg * P:(hg + 1) * P],
                                        xn2T[:, kc, th * SEG:(th + 1) * SEG],
                                        start=(kc == 0), stop=(kc == KC - 1))
                                nc.scalar.activation(
                                    dstT[:, hg, th * SEG:(th + 1) * SEG], ps[:],
                                    AF.Identity, scale=1.0, bias=bias_sb[:, hg:hg + 1])

                    for it in range(NT):
                        ps = ps_qkv.tile([P, DM], F32, tag="v_ps")
                        for kc in range(KC):
                            nc.tensor.matmul(
                                ps[:], xn2T[:, kc, it * P:(it + 1) * P], wv_sb[:, kc, :],
                                start=(kc == 0), stop=False)
                        nc.tensor.matmul(ps[:], ones_r[:], wvb_row[:],
                                         start=False, stop=True)
                        nc.scalar.copy(V_sb[:, it, :], ps[:])

                # per-head attention
                inv_sqrt = 1.0 / float(np.sqrt(DK))
                with tc.tile_pool(name="pet", bufs=3) as pet, \
                        tc.tile_pool(name="pew", bufs=2) as pew, \
                        tc.tile_pool(name="ps_av", bufs=1, space="PSUM") as ps_avp, \
                        tc.tile_pool(name="ps_hd", bufs=2, space="PSUM") as ps_hd:
                    for h in range(H):
                        hg, hh = h // 2, h % 2
                        qsl = slice(hh * DK, (hh + 1) * DK)

                        # natural scores -> E, rowsums -> W -> DRAM; recips -> DRAM
                        for iq in range(NT):
                            e_t = pew.tile([P, L], F32, tag="e_t")
                            rs = pstat.tile([P, TH], F32, tag="rs")
                            for kh in range(TH):
                                ps = ps_hd.tile([P, SEG], F32, tag="s_ps")
                                nc.tensor.matmul(
                                    ps[:], QT[qsl, hg, iq * P:(iq + 1) * P],
                                    KT[qsl, hg, kh * SEG:(kh + 1) * SEG],
                                    start=True, stop=True)
                                nc.scalar.activation(
                                    e_t[:, kh * SEG:(kh + 1) * SEG], ps[:], AF.Exp,
                                    scale=inv_sqrt, accum_out=rs[:, kh:kh + 1])
                            rsum = pstat.tile([P, 1], F32, tag="rsum")
                            if TH == 1:
                                nc.vector.reciprocal(rsum[:], rs[:, 0:1])
                            else:
                                nc.vector.tensor_add(rsum[:], rs[:, 0:1], rs[:, 1:2])
                                for kh in range(2, TH):
                                    nc.vector.tensor_add(rsum[:], rsum[:], rs[:, kh:kh + 1])
                                nc.vector.reciprocal(rsum[:], rsum[:])
                            nc.sync.dma_start(
                                recip_dram[h, iq * P:(iq + 1) * P, :], rsum[:])
                            nc.vector.tensor_scalar_mul(e_t[:], e_t[:], rsum[:])
                            nc.sync.dma_start(
                                attn_w_out[h, iq * P:(iq + 1) * P, :], e_t[:])

                        # scores^T -> exp -> ET, attn@V accumulated per th
                        rec_rep = pew.tile([P, L], F32, tag="rec_rep")
                        nc.sync.dma_start(
                            rec_rep[:],
                            recip_dram[h, :, 0][None, :].partition_broadcast(P))
                        ps_av = [ps_avp.tile([DK, SEG], F32, tag=f"av_ps{th}",
                                             name=f"av_ps{th}") for th in range(TH)]
                        for ik in range(NT):
                            et = pet.tile([P, L], F32R, tag="et")
                            for th in range(TH):
                                ps = ps_hd.tile([P, SEG], F32, tag="sT_ps")
                                nc.tensor.matmul(
                                    ps[:], KT[qsl, hg, ik * P:(ik + 1) * P],
                                    QT[qsl, hg, th * SEG:(th + 1) * SEG],
                                    start=True, stop=True)
                                nc.scalar.activation(
                                    et[:, th * SEG:(th + 1) * SEG], ps[:], AF.Exp,
                                    scale=inv_sqrt)
                            for th in range(TH):
                                nc.tensor.matmul(
                                    ps_av[th][:], V_sb[:, ik, h * DK:(h + 1) * DK],
                                    et[:, th * SEG:(th + 1) * SEG],
                                    start=(ik == 0), stop=(ik == NT - 1))
                        for th in range(TH):
                            tsl = slice(th * SEG, (th + 1) * SEG)
                            if hh == 0:
                                nc.vector.tensor_mul(
                                    attn_oT[0:DK, hg, tsl], ps_av[th][:],
                                    rec_rep[0:DK, tsl])
                            else:
                                t_av = pew.tile([DK, SEG], F32R, tag="t_av")
                                nc.vector.tensor_mul(
                                    t_av[:], ps_av[th][:], rec_rep[0:DK, tsl])
                                nc.sync.dma_start(attn_oT[DK:P, hg, tsl], t_av[:])

            # wo + residual + attn LN + gate + fused combine
            with tc.tile_pool(name="awork", bufs=2) as awork, \
                    tc.tile_pool(name="apost", bufs=1) as apost, \
                    tc.tile_pool(name="ps_wo", bufs=2, space="PSUM") as ps_wo:
                a_out = apost.tile([P, NT, DM], F32, tag="a_out")
                a_outT = apost.tile([P, KC, L], F32R, tag="a_outT")
                wo_sb = apost.tile([P, KC, DM], F32R, tag="wo_sb")
                nc.sync.dma_start(
                    wo_sb[:], wd["wo_w"][:].rearrange("(c p) o -> p c o", p=P).bitcast(F32R))
                for it in range(NT):
                    ps = ps_wo.tile([P, DM], F32, tag="wo_ps")
                    for kc in range(KC):
                        nc.tensor.matmul(
                            ps[:], attn_oT[:, kc, it * P:(it + 1) * P], wo_sb[:, kc, :],
                            start=(kc == 0), stop=False)
                    nc.tensor.matmul(ps[:], ones_r[:], wob_row[:], start=False, stop=True)
                    r2 = awork.tile([P, DM], F32, tag="r2")
                    nc.vector.tensor_add(r2[:], ps[:], xn2[:, it, :])
                    cs = center_scale(r2[:])
                    nc.vector.tensor_mul(a_out[:, it, :], cs[:], ga_rep[:])
                    nc.vector.tensor_add(a_out[:, it, :], a_out[:, it, :], ba_rep[:])
                    transpose_apply(cs[:], it, ga_sb, ba_sb, a_outT)
                    if debug:
                        nc.sync.dma_start(
                            dbg_a[:].rearrange("(n p) d -> p n d", p=P)[:, it, :],
                            a_out[:, it, :])

                gw_sb = apost.tile([P, 2 * KC, DM], F32R, tag="gw_sb")
                nc.sync.dma_start(
                    gw_sb[:], wd["gate_w"][:].rearrange("(c p) o -> p c o", p=P).bitcast(F32R))
                for it in range(NT):
                    ps = ps_wo.tile([P, DM], F32, tag="g_ps")
                    for kc in range(KC):
                        nc.tensor.matmul(
                            ps[:], m_outT[:, kc, it * P:(it + 1) * P], gw_sb[:, kc, :],
                            start=(kc == 0), stop=False)
                    for kc in range(KC):
                        nc.tensor.matmul(
                            ps[:], a_outT[:, kc, it * P:(it + 1) * P],
                            gw_sb[:, KC + kc, :], start=False, stop=False)
                    nc.tensor.matmul(ps[:], ones_r[:], gateb_row[:], start=False, stop=True)
                    g_t = awork.tile([P, DM], F32, tag="g_t")
                    nc.scalar.activation(g_t[:], ps[:], AF.Sigmoid)
                    if debug:
                        nc.sync.dma_start(
                            dbg_g[:].rearrange("(n p) d -> p n d", p=P)[:, it, :], g_t[:])
                    # x3 = x2 + fused = x1 + 2*a + g*(m - a)
                    t1 = awork.tile([P, DM], F32, tag="f_t1")
                    nc.vector.tensor_tensor(out=t1[:], in0=m_out[:, it, :],
                                            in1=a_out[:, it, :], op=OP.subtract)
                    t2 = awork.tile([P, DM], F32, tag="f_t2")
                    nc.vector.tensor_mul(t2[:], g_t[:], t1[:])
                    nc.vector.scalar_tensor_tensor(
                        out=t2[:], in0=a_out[:, it, :], scalar=2.0, in1=t2[:],
                        op0=OP.mult, op1=OP.add)
                    nc.vector.tensor_add(x_acc[:, it, :], x_acc[:, it, :], t2[:])
                    if debug:
                        nc.sync.dma_start(
                            dbg_x3[:].rearrange("(n p) d -> p n d", p=P)[:, it, :],
                            x_acc[:, it, :])

        mid_cm.__exit__(None, None, None)

        # ================= FFN =================
        with tc.tile_pool(name="ffn", bufs=1) as fpool, \
                tc.tile_pool(name="fwork", bufs=2) as fwork, \
                tc.tile_pool(name="ps_ffn", bufs=2, space="PSUM") as ps_ffn:
            xn3T = fpool.tile([P, KC, L], BF16, tag="xn3T")
            for it in range(NT):
                cs = center_scale(x_acc[:, it, :])
                for jc in range(KC):
                    ps = ptr.tile([P, P], F32, tag="tr_ps")
                    nc.tensor.transpose(ps[:], cs[:, jc * P:(jc + 1) * P], ident_f[:])
                    nc.scalar.activation(
                        xn3T[:, jc, it * P:(it + 1) * P], ps[:], AF.Identity,
                        scale=g3_sb[:, jc:jc + 1], bias=b3n_sb[:, jc:jc + 1])

            f1_sb = fpool.tile([P, KC, DFF], BF16, tag="f1_sb")
            nc.gpsimd.dma_start(
                f1_sb[:], wd["ffn_w1"][:].rearrange("(c p) o -> p c o", p=P))
            h1T = fpool.tile([P, KF, L], BF16, tag="h1T")
            for ff in range(KF):
                for th in range(TH):
                    ps = ps_ffn.tile([P, SEG], F32, tag="f1_ps")
                    for kc in range(KC):
                        nc.tensor.matmul(
                            ps[:], f1_sb[:, kc, ff * P:(ff + 1) * P],
                            xn3T[:, kc, th * SEG:(th + 1) * SEG],
                            start=(kc == 0), stop=(kc == KC - 1))
                    # gelu_tanh(x) = 0.5*x*(1+tanh(c1*x + c2*x^3)), x = psum + b1
                    x_t = fwork.tile([P, SEG], F32, tag="x_t")
                    nc.scalar.activation(x_t[:], ps[:], AF.Identity,
                                         bias=fb1_sb[:, ff:ff + 1])
                    s_t = fwork.tile([P, SEG], F32, tag="s_t")
                    nc.scalar.activation(s_t[:], x_t[:], AF.Square)
                    p_t = fwork.tile([P, SEG], F32, tag="p_t")
                    nc.vector.tensor_scalar(
                        out=p_t[:], in0=s_t[:], scalar1=0.044715 * 0.7978845608028654,
                        scalar2=0.7978845608028654, op0=OP.mult, op1=OP.add)
                    nc.vector.tensor_mul(p_t[:], p_t[:], x_t[:])
                    th_t = fwork.tile([P, SEG], F32, tag="th_t")
                    nc.scalar.activation(th_t[:], p_t[:], AF.Tanh)
                    q_t = fwork.tile([P, SEG], F32, tag="q_t")
                    nc.vector.tensor_scalar(
                        out=q_t[:], in0=th_t[:], scalar1=1.0, scalar2=0.5,
                        op0=OP.add, op1=OP.mult)
                    nc.vector.tensor_mul(
                        h1T[:, ff, th * SEG:(th + 1) * SEG], q_t[:], x_t[:])
                    if debug:
                        h1f = fwork.tile([P, SEG], F32, tag="h1f")
                        nc.vector.tensor_mul(h1f[:], q_t[:], x_t[:])
                        nc.sync.dma_start(
                            dbg_h1[:].rearrange("(f p) t -> p f t", p=P)[:, ff, th * SEG:(th + 1) * SEG],
                            h1f[:])

            f2_sb = fpool.tile([P, KF, DM], BF16, tag="f2_sb")
            nc.gpsimd.dma_start(
                f2_sb[:], wd["ffn_w2"][:].rearrange("(c p) o -> p c o", p=P))
            for it in range(NT):
                ps = ps_ffn.tile([P, DM], F32, tag="f2_ps")
                for kc in range(KF):
                    nc.tensor.matmul(
                        ps[:], h1T[:, kc, it * P:(it + 1) * P], f2_sb[:, kc, :],
                        start=(kc == 0), stop=False)
                nc.tensor.matmul(ps[:], ones_r[:], fb2_row[:], start=False, stop=True)
                o_t = fwork.tile([P, DM], F32, tag="o_t")
                nc.vector.tensor_add(o_t[:], ps[:], x_acc[:, it, :])
                nc.sync.dma_start(
                    out_x[:].rearrange("(n p) d -> p n d", p=P)[:, it, :], o_t[:])

    nc.compile()
    return nc


_CACHE = {}


def _get_program(L):
    if L not in _CACHE:
        _CACHE[L] = build_program(L)
    return _CACHE[L]


def kernel(**inputs):
    from concourse.bass_utils import run_bass_kernel_spmd

    x = np.ascontiguousarray(inputs["x"], dtype=np.float32)
    Bx, L, _ = x.shape
    nc = _get_program(L)

    weights = {}
    for name in ["in_proj_w", "conv_w", "conv_b", "x_proj_w", "dt_proj_w", "dt_proj_b",
                 "A_log", "D", "out_proj_w", "wq_w", "wq_b", "wk_w", "wk_b", "wv_w",
                 "wv_b", "wo_w", "wo_b", "attn_ln_g", "attn_ln_b", "ffn_w1", "ffn_b1",
                 "ffn_w2", "ffn_b2", "norm1_g", "norm1_b", "norm2_g", "norm2_b",
                 "norm3_g", "norm3_b", "gate_w", "gate_b"]:
        weights[name] = np.ascontiguousarray(inputs[name], dtype=np.float32)
    weights["ident"] = np.eye(P, dtype=np.float32)
    weights["ones_row"] = np.ones((1, P), dtype=np.float32)

    in_maps = [{"x_in": np.ascontiguousarray(x[b]), **weights} for b in range(Bx)]
    res = run_bass_kernel_spmd(nc, in_maps, core_ids=list(range(Bx)))
    out = np.stack([r["out_x"] for r in res.results], axis=0)
    attn_w = np.stack([r["attn_w_out"] for r in res.results], axis=0)
    return (out, attn_w)


if __name__ == "__main__":
    build_program(256)
    print("built OK")


# revision 25
# speedup vs baseline: 1.1798x; 1.1265x over previous
"""EnhancedMambaFormerBlock Trainium2 kernel.

Data-parallel over batch: 8 batch elements -> 8 NeuronCores, no collectives.
Per core the full block (LN -> Mamba(conv + selective scan) -> LN -> attention
-> gated fusion -> LN -> FFN) runs with a transposed-layout compute chain:

 - Matmuls run in fp32r (full PE speed at N>=256, ~1e-4 rounding) or bf16
   where precision allows (dt_proj / x_proj / ffn).
 - The selective scan uses the DVE `tensor_tensor_scan` instruction
   (state = dA * state + dBx along time), one scan per
   (state n, 128-channel block, 512-time segment); decay factors come from
   ScalarE exp(A_n * delta); the C-contraction over n uses GpSimd multiplies
   + PE identity-matmul accumulation in PSUM.
 - Attention scores are computed in BOTH layouts ([tq,tk] for softmax /
   attn_w output, [tk,tq] for attn@V); exp + row sums via ScalarE accum_out.

Self-contained: hardcodes all shapes from the problem spec.
"""
import sys
from contextlib import ExitStack

import numpy as np

sys.path.insert(0, "/opt/trn_rl_repo")

import concourse.tile as tile  # noqa: E402
from concourse import bacc, mybir  # noqa: E402

F32 = mybir.dt.float32
F32R = mybir.dt.float32r
BF16 = mybir.dt.bfloat16
AF = mybir.ActivationFunctionType
OP = mybir.AluOpType

DM = 512
DS = 16
DCONV = 4
DI = 1024
H = 8
DK = 64
DFF = 2048
EPS = 1e-5
P = 128

KC = DM // P   # 4  dmodel chunks
KI = DI // P   # 8  d_inner chunks
KF = DFF // P  # 16 dff chunks


def build_program(L=1024, debug=False):
    NT = L // P
    SEG = min(512, L)
    TH = L // SEG

    nc = bacc.Bacc("TRN2", target_bir_lowering=False, debug=False)

    x_in = nc.dram_tensor("x_in", [L, DM], F32, kind="ExternalInput")
    out_x = nc.dram_tensor("out_x", [L, DM], F32, kind="ExternalOutput")
    attn_w_out = nc.dram_tensor("attn_w_out", [H, L, L], F32, kind="ExternalOutput")
    if debug:
        dbg_m = nc.dram_tensor("dbg_m", [L, DM], F32, kind="ExternalOutput")
        dbg_a = nc.dram_tensor("dbg_a", [L, DM], F32, kind="ExternalOutput")
        dbg_g = nc.dram_tensor("dbg_g", [L, DM], F32, kind="ExternalOutput")
        dbg_x3 = nc.dram_tensor("dbg_x3", [L, DM], F32, kind="ExternalOutput")
        dbg_h1 = nc.dram_tensor("dbg_h1", [DFF, L], F32, kind="ExternalOutput")

    wd = {}
    for name, shape in [
        ("in_proj_w", [DM, 2 * DI]), ("conv_w", [DI, DCONV]), ("conv_b", [DI]),
        ("x_proj_w", [DI, 2 * DS]), ("dt_proj_w", [DI, DI]), ("dt_proj_b", [DI]),
        ("A_log", [DS]), ("D", [DI]), ("out_proj_w", [DI, DM]),
        ("wq_w", [DM, DM]), ("wq_b", [DM]), ("wk_w", [DM, DM]), ("wk_b", [DM]),
        ("wv_w", [DM, DM]), ("wv_b", [DM]), ("wo_w", [DM, DM]), ("wo_b", [DM]),
        ("attn_ln_g", [DM]), ("attn_ln_b", [DM]),
        ("ffn_w1", [DM, DFF]), ("ffn_b1", [DFF]), ("ffn_w2", [DFF, DM]), ("ffn_b2", [DM]),
        ("norm1_g", [DM]), ("norm1_b", [DM]), ("norm2_g", [DM]), ("norm2_b", [DM]),
        ("norm3_g", [DM]), ("norm3_b", [DM]),
        ("gate_w", [2 * DM, DM]), ("gate_b", [DM]),
        ("ident", [P, P]), ("ones_row", [1, P]),
    ]:
        wd[name] = nc.dram_tensor(name, shape, F32, kind="ExternalInput")

    with tile.TileContext(nc) as tc, ExitStack() as top:
        persist = top.enter_context(tc.tile_pool(name="persist", bufs=1))
        dscratch = top.enter_context(tc.tile_pool(name="dscratch", bufs=1, space="DRAM"))
        pwork = top.enter_context(tc.tile_pool(name="pwork", bufs=2))
        pstat = top.enter_context(tc.tile_pool(name="pstat", bufs=4))
        ptr = top.enter_context(tc.tile_pool(name="ptr", bufs=2, space="PSUM"))

        # ---------------- consts ----------------
        ident_f = persist.tile([P, P], F32, tag="ident_f")
        nc.sync.dma_start(ident_f[:], wd["ident"][:])
        ident_r = persist.tile([P, P], F32R, tag="ident_r")
        nc.sync.dma_start(ident_r[:], wd["ident"][:].bitcast(F32R))
        ones_r = persist.tile([1, P], F32R, tag="ones_r")
        nc.sync.dma_start(ones_r[:], wd["ones_row"][:].bitcast(F32R))
        ident_b = persist.tile([P, P], BF16, tag="ident_b")
        nc.gpsimd.dma_start(ident_b[:], wd["ident"][:])
        eps_t = persist.tile([P, 1], F32, tag="eps")
        nc.vector.memset(eps_t[:], EPS)

        alog_rep = persist.tile([P, DS], F32, tag="alog_rep")
        nc.sync.dma_start(alog_rep[:], wd["A_log"][None, :].partition_broadcast(P))
        A_rep = persist.tile([P, DS], F32, tag="A_rep")
        nc.scalar.activation(A_rep[:], alog_rep[:], AF.Exp)
        nc.vector.tensor_scalar_mul(A_rep[:], A_rep[:], -1.0)

        def ppart(name, cols, tag):
            t = persist.tile([P, cols], F32, tag=tag)
            nc.sync.dma_start(t[:], wd[name][:].rearrange("(o p) -> p o", p=P))
            return t

        conv_b_sb = ppart("conv_b", KI, "conv_b")
        dt_b_sb = ppart("dt_proj_b", KI, "dt_b")
        D_sb = ppart("D", KI, "D_sb")
        g1_sb = ppart("norm1_g", KC, "g1")
        b1n_sb = ppart("norm1_b", KC, "b1n")
        g2_sb = ppart("norm2_g", KC, "g2")
        b2n_sb = ppart("norm2_b", KC, "b2n")
        g3_sb = ppart("norm3_g", KC, "g3")
        b3n_sb = ppart("norm3_b", KC, "b3n")
        ga_sb = ppart("attn_ln_g", KC, "ga")
        ba_sb = ppart("attn_ln_b", KC, "ba")
        wqb_sb = ppart("wq_b", KC, "wqb")
        wkb_sb = ppart("wk_b", KC, "wkb")
        fb1_sb = ppart("ffn_b1", KF, "fb1")
        conv_w_sb = persist.tile([P, KI, DCONV], F32, tag="conv_w")
        nc.sync.dma_start(conv_w_sb[:], wd["conv_w"][:].rearrange("(o p) k -> p o k", p=P))

        def vrep(name, tag):
            t = persist.tile([P, DM], F32, tag=tag)
            nc.sync.dma_start(t[:], wd[name][None, :].partition_broadcast(P))
            return t

        g2_rep = vrep("norm2_g", "g2rep")
        b2_rep = vrep("norm2_b", "b2rep")
        ga_rep = vrep("attn_ln_g", "garep")
        ba_rep = vrep("attn_ln_b", "barep")

        def brow(name, n, tag):
            t = persist.tile([1, n], F32R, tag=tag)
            nc.sync.dma_start(t[:], wd[name][None, :].bitcast(F32R))
            return t

        wvb_row = brow("wv_b", DM, "wvb_row")
        wob_row = brow("wo_b", DM, "wob_row")
        gateb_row = brow("gate_b", DM, "gateb_row")
        fb2_row = brow("ffn_b2", DM, "fb2_row")

        recip_dram = dscratch.tile([H, L, 1], F32, tag="recip_dram")
        x_acc = persist.tile([P, NT, DM], F32, tag="x_acc")
        nc.sync.dma_start(x_acc[:], x_in[:].rearrange("(n p) d -> p n d", p=P))



        # ---------------- helpers ----------------
        def ln_stats(xt):
            st = pstat.tile([P, 6], F32, tag="st6")
            nc.vector.bn_stats(st[:], xt)
            mv = pstat.tile([P, 2], F32, tag="mv2")
            nc.vector.bn_aggr(mv[:], st[:])
            rstd = pstat.tile([P, 1], F32, tag="rstd")
            nc.scalar.activation(rstd[:], mv[:, 1:2], AF.Sqrt, bias=eps_t[:])
            nc.vector.reciprocal(rstd[:], rstd[:])
            return mv[:, 0:1], rstd

        def center_scale(src):
            mu, rstd = ln_stats(src)
            cs = pwork.tile([P, DM], F32, tag="cs_t")
            nc.vector.tensor_scalar(
                out=cs[:], in0=src, scalar1=mu, scalar2=rstd,
                op0=OP.subtract, op1=OP.mult)
            return cs

        def transpose_apply(cs_t, it, g_col, b_col, dstT):
            for jc in range(KC):
                ps = ptr.tile([P, P], F32, tag="tr_ps")
                nc.tensor.transpose(ps[:], cs_t[:, jc * P:(jc + 1) * P], ident_f[:])
                nc.scalar.activation(
                    dstT[:, jc, it * P:(it + 1) * P], ps[:], AF.Identity,
                    scale=g_col[:, jc:jc + 1], bias=b_col[:, jc:jc + 1])

        # ================= mamba =================
        mid_cm = tc.tile_pool(name="mid", bufs=1)
        mid = mid_cm.__enter__()
        m_outT = mid.tile([P, KC, L], F32R, tag="m_outT")
        m_out = mid.tile([P, NT, DM], F32, tag="m_out")
        with tc.tile_pool(name="mamba", bufs=1) as mpool:
            xcT = mpool.tile([P, KI, L], BF16, tag="xcT")
            BCT_dram = dscratch.tile([2 * DS, L], F32, tag="BCT_dram")
            sresT = mpool.tile([P, KI, L], BF16, tag="sresT")
            BCT = mpool.tile([2 * DS, L], F32, tag="BCT")

            with tc.tile_pool(name="inproj", bufs=1) as ipool, \
                    tc.tile_pool(name="iwork", bufs=2) as iwork, \
                    tc.tile_pool(name="ps_ip", bufs=2, space="PSUM") as ps_ip:
                xn1T = ipool.tile([P, KC, L], F32R, tag="xn1T")
                for it in range(NT):
                    cs = center_scale(x_acc[:, it, :])
                    transpose_apply(cs[:], it, g1_sb, b1n_sb, xn1T)

                xmT_pad = ipool.tile([P, KI, L + DCONV - 1], F32, tag="xmT_pad")
                for oi in range(KI):
                    nc.vector.memset(xmT_pad[:, oi, 0:DCONV - 1], 0.0)

                for half in range(2):
                    ipw = ipool.tile([P, KC, DI], F32R, tag="ipw")
                    nc.sync.dma_start(
                        ipw[:], wd["in_proj_w"][:, half * DI:(half + 1) * DI]
                        .rearrange("(c p) o -> p c o", p=P).bitcast(F32R))
                    for oi in range(KI):
                        for th in range(TH):
                            ps = ps_ip.tile([P, SEG], F32, tag="ip_ps")
                            for kc in range(KC):
                                nc.tensor.matmul(
                                    ps[:], ipw[:, kc, oi * P:(oi + 1) * P],
                                    xn1T[:, kc, th * SEG:(th + 1) * SEG],
                                    start=(kc == 0), stop=(kc == KC - 1))
                            if half == 0:
                                nc.scalar.copy(
                                    xmT_pad[:, oi, DCONV - 1 + th * SEG:DCONV - 1 + (th + 1) * SEG],
                                    ps[:])
                            else:
                                sg = iwork.tile([P, SEG], F32, tag="sg_t")
                                nc.scalar.activation(sg[:], ps[:], AF.Sigmoid)
                                nc.vector.tensor_mul(
                                    sresT[:, oi, th * SEG:(th + 1) * SEG], ps[:], sg[:])

                # causal depthwise conv + silu -> xcT
                for oi in range(KI):
                    acc = iwork.tile([P, L], F32, tag="cv0")
                    nc.vector.scalar_tensor_tensor(
                        out=acc[:], in0=xmT_pad[:, oi, 0:L], scalar=conv_w_sb[:, oi, 0:1],
                        in1=conv_b_sb[:, oi:oi + 1].to_broadcast([P, L]),
                        op0=OP.mult, op1=OP.add)
                    for k in range(1, DCONV):
                        nc.vector.scalar_tensor_tensor(
                            out=acc[:], in0=xmT_pad[:, oi, k:k + L],
                            scalar=conv_w_sb[:, oi, k:k + 1], in1=acc[:],
                            op0=OP.mult, op1=OP.add)
                    for th in range(TH):
                        tsl = slice(th * SEG, (th + 1) * SEG)
                        sg2 = iwork.tile([P, SEG], F32, tag="sg2_t")
                        nc.scalar.activation(sg2[:], acc[:, tsl], AF.Sigmoid)
                        nc.vector.tensor_mul(xcT[:, oi, tsl], acc[:, tsl], sg2[:])

                # x_proj -> BCT [2*DS, L]
                xpw = ipool.tile([P, KI, 2 * DS], BF16, tag="xpw")
                nc.gpsimd.dma_start(
                    xpw[:], wd["x_proj_w"][:].rearrange("(c p) s -> p c s", p=P))
                for th in range(TH):
                    ps = ps_ip.tile([2 * DS, SEG], F32, tag="xp_ps")
                    for kc in range(KI):
                        nc.tensor.matmul(
                            ps[:], xpw[:, kc, :], xcT[:, kc, th * SEG:(th + 1) * SEG],
                            start=(kc == 0), stop=(kc == KI - 1))
                    nc.scalar.copy(BCT[:, th * SEG:(th + 1) * SEG], ps[:])
                nc.sync.dma_start(BCT_dram[:], BCT[:])

            # ---------- selective scan ----------
            hcarry = mpool.tile([P, KI, DS], F32, tag="hcarry")

            with tc.tile_pool(name="sweights", bufs=1) as swp, \
                    tc.tile_pool(name="mwork", bufs=2) as mwork, \
                    tc.tile_pool(name="shot", bufs=4) as shot, \
                    tc.tile_pool(name="pbc", bufs=1) as pbc, \
                    tc.tile_pool(name="ps_scan", bufs=1, space="PSUM") as ps_scan:
                opw = swp.tile([P, KI, DM], F32R, tag="opw")
                nc.sync.dma_start(
                    opw[:], wd["out_proj_w"][:].rearrange("(c p) o -> p c o", p=P).bitcast(F32R))
                for th in range(TH):
                    sl = slice(th * SEG, (th + 1) * SEG)
                    B_rep = pbc.tile([P, DS, SEG], BF16, tag="B_rep")
                    C_rep = pbc.tile([P, DS, SEG], BF16, tag="C_rep")
                    for n in range(DS):
                        nc.gpsimd.dma_start(
                            B_rep[:, n, :], BCT_dram[n:n + 1, sl].partition_broadcast(P))
                        nc.gpsimd.dma_start(
                            C_rep[:, n, :], BCT_dram[DS + n:DS + n + 1, sl].partition_broadcast(P))

                    psum_op = [ps_scan.tile([P, SEG], F32, tag=f"op_ps{mt}",
                                            name=f"op_ps{mt}") for mt in range(KC)]

                    for oi in range(KI):
                        dtw = mwork.tile([P, KI, P], BF16, tag="dtw")
                        nc.gpsimd.dma_start(
                            dtw[:], wd["dt_proj_w"][:, oi * P:(oi + 1) * P]
                            .rearrange("(c p) o -> p c o", p=P))
                        ps_dt = ps_scan.tile([P, SEG], F32, tag="dt_ps")
                        for kc in range(KI):
                            nc.tensor.matmul(ps_dt[:], dtw[:, kc, :], xcT[:, kc, sl],
                                             start=(kc == 0), stop=(kc == KI - 1))
                        # softplus(z) = ln(exp(z) + 1), z = psum + dt_b
                        delta = mwork.tile([P, SEG], F32, tag="delta")
                        nc.scalar.activation(delta[:], ps_dt[:], AF.Exp,
                                             bias=dt_b_sb[:, oi:oi + 1])
                        nc.scalar.activation(delta[:], delta[:], AF.Ln, bias=1.0)
                        du = mwork.tile([P, SEG], BF16, tag="du")
                        nc.vector.tensor_mul(du[:], delta[:], xcT[:, oi, sl])

                        ps_y = ps_scan.tile([P, SEG], F32, tag="y_ps")
                        for n in range(DS):
                            dA = shot.tile([P, SEG], BF16, tag="dA")
                            nc.scalar.activation(dA[:], delta[:], AF.Exp,
                                                 scale=A_rep[:, n:n + 1])
                            dBx = shot.tile([P, SEG], BF16, tag="dBx")
                            nc.vector.tensor_mul(dBx[:], B_rep[:, n, :], du[:])
                            h = shot.tile([P, SEG], BF16, tag="h")
                            init = 0.0 if th == 0 else hcarry[:, oi, n:n + 1]
                            nc.vector.tensor_tensor_scan(
                                out=h[:], data0=dA[:], data1=dBx[:], initial=init,
                                op0=OP.mult, op1=OP.add)
                            if th != TH - 1:
                                nc.gpsimd.tensor_copy(hcarry[:, oi, n:n + 1],
                                                      h[:, SEG - 1:SEG])
                            ch = shot.tile([P, SEG], BF16, tag="ch")
                            if n % 3 == 0:
                                nc.vector.tensor_tensor(out=ch[:], in0=h[:],
                                                        in1=C_rep[:, n, :], op=OP.mult)
                            else:
                                nc.gpsimd.tensor_tensor(out=ch[:], in0=h[:],
                                                        in1=C_rep[:, n, :], op=OP.mult)
                            nc.tensor.matmul(ps_y[:], ident_b[:], ch[:],
                                             start=(n == 0), stop=(n == DS - 1))

                        t1 = mwork.tile([P, SEG], F32, tag="t1")
                        nc.vector.scalar_tensor_tensor(
                            out=t1[:], in0=xcT[:, oi, sl], scalar=D_sb[:, oi:oi + 1],
                            in1=ps_y[:], op0=OP.mult, op1=OP.add)
                        yg = mwork.tile([P, SEG], F32R, tag="yg")
                        nc.vector.tensor_mul(yg[:], t1[:], sresT[:, oi, sl])

                        for mt in range(KC):
                            nc.tensor.matmul(
                                psum_op[mt][:], opw[:, oi, mt * P:(mt + 1) * P], yg[:],
                                start=(oi == 0), stop=(oi == KI - 1))

                    for mt in range(KC):
                        nc.scalar.copy(m_outT[:, mt, sl], psum_op[mt][:])

        # mamba_out natural + x1 = x + mamba_out
        for it in range(NT):
            for jc in range(KC):
                ps = ptr.tile([P, P], F32, tag="tr_ps")
                nc.tensor.transpose(
                    ps[:], m_outT[:, jc, it * P:(it + 1) * P].bitcast(F32), ident_f[:])
                nc.scalar.copy(m_out[:, it, jc * P:(jc + 1) * P], ps[:])
            nc.vector.tensor_add(x_acc[:, it, :], x_acc[:, it, :], m_out[:, it, :])
            if debug:
                nc.sync.dma_start(
                    dbg_m[:].rearrange("(n p) d -> p n d", p=P)[:, it, :], m_out[:, it, :])

        # ================= attention =================
        with tc.tile_pool(name="attn", bufs=1) as apool:
            xn2 = apool.tile([P, NT, DM], F32, tag="xn2")
            attn_oT = apool.tile([P, KC, L], F32R, tag="attn_oT")

            with tc.tile_pool(name="qk", bufs=1) as qpool:
                QT = qpool.tile([P, KC, L], F32R, tag="QT")
                KT = qpool.tile([P, KC, L], F32R, tag="KT")
                V_sb = qpool.tile([P, NT, DM], F32R, tag="V_sb")

                with tc.tile_pool(name="wqkv", bufs=1) as wpool, \
                        tc.tile_pool(name="ps_qkv", bufs=2, space="PSUM") as ps_qkv:
                    xn2T = wpool.tile([P, KC, L], F32R, tag="xn2T")
                    for it in range(NT):
                        cs = center_scale(x_acc[:, it, :])
                        nc.vector.tensor_mul(xn2[:, it, :], cs[:], g2_rep[:])
                        nc.vector.tensor_add(xn2[:, it, :], xn2[:, it, :], b2_rep[:])
                        transpose_apply(cs[:], it, g2_sb, b2n_sb, xn2T)

                    wq_sb = wpool.tile([P, KC, DM], F32R, tag="wq_sb")
                    nc.sync.dma_start(
                        wq_sb[:], wd["wq_w"][:].rearrange("(c p) o -> p c o", p=P).bitcast(F32R))
                    wk_sb = wpool.tile([P, KC, DM], F32R, tag="wk_sb")
                    nc.sync.dma_start(
                        wk_sb[:], wd["wk_w"][:].rearrange("(c p) o -> p c o", p=P).bitcast(F32R))
                    wv_sb = wpool.tile([P, KC, DM], F32R, tag="wv_sb")
                    nc.sync.dma_start(
                        wv_sb[:], wd["wv_w"][:].rearrange("(c p) o -> p c o", p=P).bitcast(F32R))

                    for hg in range(KC):
                        for th in range(TH):
                            for (w_sb, bias_sb, dstT) in (
                                    (wq_sb, wqb_sb, QT), (wk_sb, wkb_sb, KT)):
                                ps = ps_qkv.tile([P, SEG], F32, tag="qk_ps")
                                for kc in range(KC):
                                    nc.tensor.matmul(
                                        ps[:], w_sb[:, kc, hg * P:(hg + 1) * P],
                                        xn2T[:, kc, th * SEG:(th + 1) * SEG],
                                        start=(kc == 0), stop=(kc == KC - 1))
                                nc.vector.tensor_scalar_add(
                                    dstT[:, hg, th * SEG:(th + 1) * SEG], ps[:],
                                    bias_sb[:, hg:hg + 1])

                    for it in range(NT):
                        ps = ps_qkv.tile([P, DM], F32, tag="v_ps")
                        for kc in range(KC):
                            nc.tensor.matmul(
                                ps[:], xn2T[:, kc, it * P:(it + 1) * P], wv_sb[:, kc, :],
                                start=(kc == 0), stop=False)
                        nc.tensor.matmul(ps[:], ones_r[:], wvb_row[:],
                                         start=False, stop=True)
                        nc.scalar.copy(V_sb[:, it, :], ps[:])

                # per-head attention
                inv_sqrt = 1.0 / float(np.sqrt(DK))
                with tc.tile_pool(name="pet", bufs=4) as pet, \
                        tc.tile_pool(name="pew", bufs=3) as pew, \
                        tc.tile_pool(name="ps_av", bufs=1, space="PSUM") as ps_avp, \
                        tc.tile_pool(name="ps_hd", bufs=2, space="PSUM") as ps_hd:
                    for h in range(H):
                        hg, hh = h // 2, h % 2
                        qsl = slice(hh * DK, (hh + 1) * DK)

                        # natural scores -> E, rowsums -> W -> DRAM; recips -> DRAM
                        for iq in range(NT):
                            e_t = pew.tile([P, L], F32, tag="e_t")
                            rs = pstat.tile([P, TH], F32, tag="rs")
                            for kh in range(TH):
                                ps = ps_hd.tile([P, SEG], F32, tag="s_ps")
                                nc.tensor.matmul(
                                    ps[:], QT[qsl, hg, iq * P:(iq + 1) * P],
                                    KT[qsl, hg, kh * SEG:(kh + 1) * SEG],
                                    start=True, stop=True)
                                nc.scalar.activation(
                                    e_t[:, kh * SEG:(kh + 1) * SEG], ps[:], AF.Exp,
                                    scale=inv_sqrt, accum_out=rs[:, kh:kh + 1])
                            rsum = pstat.tile([P, 1], F32, tag="rsum")
                            if TH == 1:
                                nc.vector.reciprocal(rsum[:], rs[:, 0:1])
                            else:
                                nc.vector.tensor_add(rsum[:], rs[:, 0:1], rs[:, 1:2])
                                for kh in range(2, TH):
                                    nc.vector.tensor_add(rsum[:], rsum[:], rs[:, kh:kh + 1])
                                nc.vector.reciprocal(rsum[:], rsum[:])
                            nc.sync.dma_start(
                                recip_dram[h, iq * P:(iq + 1) * P, :], rsum[:])
                            nc.vector.tensor_scalar_mul(e_t[:], e_t[:], rsum[:])
                            nc.sync.dma_start(
                                attn_w_out[h, iq * P:(iq + 1) * P, :], e_t[:])

                        # scores^T -> exp -> ET, attn@V accumulated per th
                        rec_rep = pew.tile([P, L], F32, tag="rec_rep")
                        nc.sync.dma_start(
                            rec_rep[:],
                            recip_dram[h, :, 0][None, :].partition_broadcast(P))
                        ps_av = [ps_avp.tile([DK, SEG], F32, tag=f"av_ps{th}",
                                             name=f"av_ps{th}") for th in range(TH)]
                        for ik in range(NT):
                            et = pet.tile([P, L], F32R, tag="et")
                            for th in range(TH):
                                ps = ps_hd.tile([P, SEG], F32, tag="sT_ps")
                                nc.tensor.matmul(
                                    ps[:], KT[qsl, hg, ik * P:(ik + 1) * P],
                                    QT[qsl, hg, th * SEG:(th + 1) * SEG],
                                    start=True, stop=True)
                                nc.scalar.activation(
                                    et[:, th * SEG:(th + 1) * SEG], ps[:], AF.Exp,
                                    scale=inv_sqrt)
                            for th in range(TH):
                                nc.tensor.matmul(
                                    ps_av[th][:], V_sb[:, ik, h * DK:(h + 1) * DK],
                                    et[:, th * SEG:(th + 1) * SEG],
                                    start=(ik == 0), stop=(ik == NT - 1))
                        for th in range(TH):
                            tsl = slice(th * SEG, (th + 1) * SEG)
                            if hh == 0:
                                nc.vector.tensor_mul(
                                    attn_oT[0:DK, hg, tsl], ps_av[th][:],
                                    rec_rep[0:DK, tsl])
                            else:
                                t_av = pew.tile([DK, SEG], F32R, tag="t_av")
                                nc.vector.tensor_mul(
                                    t_av[:], ps_av[th][:], rec_rep[0:DK, tsl])
                                nc.sync.dma_start(attn_oT[DK:P, hg, tsl], t_av[:])

            # wo + residual + attn LN + gate + fused combine
            with tc.tile_pool(name="awork", bufs=2) as awork, \
                    tc.tile_pool(name="apost", bufs=1) as apost, \
                    tc.tile_pool(name="ps_wo", bufs=2, space="PSUM") as ps_wo:
                a_out = apost.tile([P, NT, DM], F32, tag="a_out")
                a_outT = apost.tile([P, KC, L], F32R, tag="a_outT")
                wo_sb = apost.tile([P, KC, DM], F32R, tag="wo_sb")
                nc.sync.dma_start(
                    wo_sb[:], wd["wo_w"][:].rearrange("(c p) o -> p c o", p=P).bitcast(F32R))
                for it in range(NT):
                    ps = ps_wo.tile([P, DM], F32, tag="wo_ps")
                    for kc in range(KC):
                        nc.tensor.matmul(
                            ps[:], attn_oT[:, kc, it * P:(it + 1) * P], wo_sb[:, kc, :],
                            start=(kc == 0), stop=False)
                    nc.tensor.matmul(ps[:], ones_r[:], wob_row[:], start=False, stop=True)
                    r2 = awork.tile([P, DM], F32, tag="r2")
                    nc.vector.tensor_add(r2[:], ps[:], xn2[:, it, :])
                    cs = center_scale(r2[:])
                    nc.vector.tensor_mul(a_out[:, it, :], cs[:], ga_rep[:])
                    nc.vector.tensor_add(a_out[:, it, :], a_out[:, it, :], ba_rep[:])
                    transpose_apply(cs[:], it, ga_sb, ba_sb, a_outT)
                    if debug:
                        nc.sync.dma_start(
                            dbg_a[:].rearrange("(n p) d -> p n d", p=P)[:, it, :],
                            a_out[:, it, :])

                gw_sb = apost.tile([P, 2 * KC, DM], F32R, tag="gw_sb")
                nc.sync.dma_start(
                    gw_sb[:], wd["gate_w"][:].rearrange("(c p) o -> p c o", p=P).bitcast(F32R))
                for it in range(NT):
                    ps = ps_wo.tile([P, DM], F32, tag="g_ps")
                    for kc in range(KC):
                        nc.tensor.matmul(
                            ps[:], m_outT[:, kc, it * P:(it + 1) * P], gw_sb[:, kc, :],
                            start=(kc == 0), stop=False)
                    for kc in range(KC):
                        nc.tensor.matmul(
                            ps[:], a_outT[:, kc, it * P:(it + 1) * P],
                            gw_sb[:, KC + kc, :], start=False, stop=False)
                    nc.tensor.matmul(ps[:], ones_r[:], gateb_row[:], start=False, stop=True)
                    g_t = awork.tile([P, DM], F32, tag="g_t")
                    nc.scalar.activation(g_t[:], ps[:], AF.Sigmoid)
                    if debug:
                        nc.sync.dma_start(
                            dbg_g[:].rearrange("(n p) d -> p n d", p=P)[:, it, :], g_t[:])
                    # x3 = x2 + fused = x1 + 2*a + g*(m - a)
                    t1 = awork.tile([P, DM], F32, tag="f_t1")
                    nc.vector.tensor_tensor(out=t1[:], in0=m_out[:, it, :],
                                            in1=a_out[:, it, :], op=OP.subtract)
                    t2 = awork.tile([P, DM], F32, tag="f_t2")
                    nc.vector.tensor_mul(t2[:], g_t[:], t1[:])
                    nc.vector.scalar_tensor_tensor(
                        out=t2[:], in0=a_out[:, it, :], scalar=2.0, in1=t2[:],
                        op0=OP.mult, op1=OP.add)
                    nc.vector.tensor_add(x_acc[:, it, :], x_acc[:, it, :], t2[:])
                    if debug:
                        nc.sync.dma_start(
                            dbg_x3[:].rearrange("(n p) d -> p n d", p=P)[:, it, :],
                            x_acc[:, it, :])

        mid_cm.__exit__(None, None, None)

        # ================= FFN =================
        with tc.tile_pool(name="ffn", bufs=1) as fpool, \
                tc.tile_pool(name="fwork", bufs=2) as fwork, \
                tc.tile_pool(name="ps_ffn", bufs=2, space="PSUM") as ps_ffn:
            xn3T = fpool.tile([P, KC, L], BF16, tag="xn3T")
            for it in range(NT):
                cs = center_scale(x_acc[:, it, :])
                for jc in range(KC):
                    ps = ptr.tile([P, P], F32, tag="tr_ps")
                    nc.tensor.transpose(ps[:], cs[:, jc * P:(jc + 1) * P], ident_f[:])
                    nc.scalar.activation(
                        xn3T[:, jc, it * P:(it + 1) * P], ps[:], AF.Identity,
                        scale=g3_sb[:, jc:jc + 1], bias=b3n_sb[:, jc:jc + 1])

            f1_sb = fpool.tile([P, KC, DFF], BF16, tag="f1_sb")
            nc.gpsimd.dma_start(
                f1_sb[:], wd["ffn_w1"][:].rearrange("(c p) o -> p c o", p=P))
            h1T = fpool.tile([P, KF, L], BF16, tag="h1T")
            for ff in range(KF):
                for th in range(TH):
                    ps = ps_ffn.tile([P, SEG], F32, tag="f1_ps")
                    for kc in range(KC):
                        nc.tensor.matmul(
                            ps[:], f1_sb[:, kc, ff * P:(ff + 1) * P],
                            xn3T[:, kc, th * SEG:(th + 1) * SEG],
                            start=(kc == 0), stop=(kc == KC - 1))
                    # gelu_tanh(x) = 0.5*x*(1+tanh(c1*x + c2*x^3)), x = psum + b1
                    x_t = fwork.tile([P, SEG], F32, tag="x_t")
                    nc.scalar.activation(x_t[:], ps[:], AF.Identity,
                                         bias=fb1_sb[:, ff:ff + 1])
                    s_t = fwork.tile([P, SEG], F32, tag="s_t")
                    nc.scalar.activation(s_t[:], x_t[:], AF.Square)
                    p_t = fwork.tile([P, SEG], F32, tag="p_t")
                    nc.vector.tensor_scalar(
                        out=p_t[:], in0=s_t[:], scalar1=0.044715 * 0.7978845608028654,
                        scalar2=0.7978845608028654, op0=OP.mult, op1=OP.add)
                    nc.vector.tensor_mul(p_t[:], p_t[:], x_t[:])
                    th_t = fwork.tile([P, SEG], F32, tag="th_t")
                    nc.scalar.activation(th_t[:], p_t[:], AF.Tanh)
                    q_t = fwork.tile([P, SEG], F32, tag="q_t")
                    nc.vector.tensor_scalar(
                        out=q_t[:], in0=th_t[:], scalar1=1.0, scalar2=0.5,
                        op0=OP.add, op1=OP.mult)
                    nc.vector.tensor_mul(
                        h1T[:, ff, th * SEG:(th + 1) * SEG], q_t[:], x_t[:])
                    if debug:
                        h1f = fwork.tile([P, SEG], F32, tag="h1f")
                        nc.vector.tensor_mul(h1f[:], q_t[:], x_t[:])
                        nc.sync.dma_start(
                            dbg_h1[:].rearrange("(f p) t -> p f t", p=P)[:, ff, th * SEG:(th + 1) * SEG],
                            h1f[:])

            f2_sb = fpool.tile([P, KF, DM], BF16, tag="f2_sb")
            nc.gpsimd.dma_start(
                f2_sb[:], wd["ffn_w2"][:].rearrange("(c p) o -> p c o", p=P))
            for it in range(NT):
                ps = ps_ffn.tile([P, DM], F32, tag="f2_ps")
                for kc in range(KF):
                    nc.tensor.matmul(
                        ps[:], h1T[:, kc, it * P:(it + 1) * P], f2_sb[:, kc, :],
                        start=(kc == 0), stop=False)
                nc.tensor.matmul(ps[:], ones_r[:], fb2_row[:], start=False, stop=True)
                o_t = fwork.tile([P, DM], F32, tag="o_t")
                nc.vector.tensor_add(o_t[:], ps[:], x_acc[:, it, :])
                nc.sync.dma_start(
                    out_x[:].rearrange("(n p) d -> p n d", p=P)[:, it, :], o_t[:])

    nc.compile()
    return nc


_CACHE = {}


def _get_program(L):
    if L not in _CACHE:
        _CACHE[L] = build_program(L)
    return _CACHE[L]


def kernel(**inputs):
    from concourse.bass_utils import run_bass_kernel_spmd

    x = np.ascontiguousarray(inputs["x"], dtype=np.float32)
    Bx, L, _ = x.shape
    nc = _get_program(L)

    weights = {}
    for name in ["in_proj_w", "conv_w", "conv_b", "x_proj_w", "dt_proj_w", "dt_proj_b",
                 "A_log", "D", "out_proj_w", "wq_w", "wq_b", "wk_w", "wk_b", "wv_w",
                 "wv_b", "wo_w", "wo_b", "attn_ln_g", "attn_ln_b", "ffn_w1", "ffn_b1",
                 "ffn_w2", "ffn_b2", "norm1_g", "norm1_b", "norm2_g", "norm2_b",
                 "norm3_g", "norm3_b", "gate_w", "gate_b"]:
        weights[name] = np.ascontiguousarray(inputs[name], dtype=np.float32)
    weights["ident"] = np.eye(P, dtype=np.float32)
    weights["ones_row"] = np.ones((1, P), dtype=np.float32)

    in_maps = [{"x_in": np.ascontiguousarray(x[b]), **weights} for b in range(Bx)]
    res = run_bass_kernel_spmd(nc, in_maps, core_ids=list(range(Bx)))
    out = np.stack([r["out_x"] for r in res.results], axis=0)
    attn_w = np.stack([r["attn_w_out"] for r in res.results], axis=0)
    return (out, attn_w)


if __name__ == "__main__":
    build_program(256)
    print("built OK")
